# revision 15
# baseline (speedup 1.0000x reference)
"""nn_Decoder (LSTM decoder) Trainium2 Bass kernel, 8-core tensor-parallel,
two phase-shifted batch streams.

Strategy (hardcoded for B=64, L=128, H=1024, O=1, T=256, 8 cores):
  The 4H=4096 gate rows are sharded 8 ways: each core owns a 128-row H-slice
  of each gate (layout f|g|i|o), computes gates transposed on PE (W_hh^T
  blocks stationary in bf16, h^T streamed), does the cell elementwise on
  ACT/DVE, and broadcasts its h^T chunk to all peers each step via one
  8-destination remote_dma_broadcast.

  The batch is split into two independent 32-wide streams (A = batch 0:32,
  B = 32:64) running the same recurrence phase-shifted by ~half a step.
  While stream A's h-broadcast is in flight, the engines process stream B's
  matmuls / cell, and vice versa.  Both streams' broadcast frames share
  SWDGE queue 0 in strict A,B alternation (a second SWDGE queue silently
  corrupts transfers on this runtime).

  v2 changes vs the 1.60 ms baseline (trace-driven):
  - The baseline ran the PE at K=4/8 HAM throttle (1.2 GHz) for 97% of the
    kernel: one >3.4us PE-idle window during the round-0/1 pipeline fill
    re-throttled it, and the steady state never has a 3.4us contiguous
    busy stretch to re-warm.  Rounds 0-1 now run big filler sections
    (FILL_EARLY) so the PE stays continuously busy through the pipeline
    fill and keeps the 2.4 GHz clock it earned during phase-1 matmuls.
  - x_gates is computed TRANSPOSED in phase 1b ([128 gate-rows, 64 batch]
    per gate block) and re-injected each round with a single N=128 matmul
    per (stream, hi/lo) using a constant identity stationary: 4 LDW+MM
    pairs per round instead of 16.
  - One [128,128] sigmoid per stream per round instead of 2x [128,64]
    (ACT ops are ~250ns fixed overhead + ~1 col/cycle).

  Other tricks carried over from the baseline:
  - Per-SOURCE arrival semaphores: each receiver's PE consumes h chunks as
    they arrive instead of waiting for all 8.
  - g-gate tanh folded into the gate sigmoid (host scales g rows by 2,
    DVE fixes up with *2-1).
  - Cell state + temporaries in SBUF.
  - x_gates re-injected into PSUM via identity matmuls (bf16 hi+lo split).
"""

import numpy as np
import ml_dtypes

B, L, H, O, T = 64, 128, 1024, 1, 256
NC = 8
NPH = 4
# device gate-column order f|g|i|o (indices into pytorch's i,f,g,o row blocks)
GATE_ORDER = [1, 2, 0, 3]
# per-stream gate slices in the [128, 128] gates tile (4 gates x 32 batch)
SG_F = slice(0, 32)
SG_G = slice(32, 64)
SG_I = slice(64, 96)
SG_O = slice(96, 128)
# Filler matmuls per stream section: keep the PE the (slightly) binding
# resource so rounds run PE-paced in lockstep across cores, and never let a
# PE-idle window reach the ~3.4us HAM MID threshold.
FILLER_N = 256
FILL_A = 2
FILL_B = 2
# Rounds 0-1 (pipeline fill) get large filler sections: the first h
# broadcasts take a full serial chain, and a single >3.4us PE-idle window
# there re-throttles the PE to 1.2 GHz for the whole kernel.
FILL_EARLY = 6

_cache = {}


def _build_lstm_nc(T_steps=T, solo=False, detect_races=True):
    import concourse.bacc as bacc
    import concourse.bass as bass
    import concourse.mybir as mybir

    dt = mybir.dt
    AF = mybir.ActivationFunctionType
    ALU = mybir.AluOpType
    Tn = T_steps

    nc = bacc.Bacc(
        None,
        target_bir_lowering=False,
        debug=False,
        num_devices=NC,
        detect_race_conditions=detect_races,
    )

    # all fp32 inputs packed as one [128, 5196] blob: latT|WlinT|blinT|WihT|bgT
    d_f32 = nc.dram_tensor("f32blob", [128, 5196], dt.float32, kind="ExternalInput")
    # all bf16 inputs packed as one [128, 4225] blob: WhT|wout|I128
    d_b16 = nc.dram_tensor("b16blob", [128, 4225], dt.bfloat16, kind="ExternalInput")
    d_out = nc.dram_tensor("outp", [64, Tn], dt.float32, kind="ExternalOutput")
    N_IN = 2

    s_f32 = nc.alloc_sbuf_tensor("s_f32", [128, 5196], dt.float32)
    s_latT = s_f32[:, 0:64]
    s_WlinT = s_f32[:, 64:1088]
    s_blinT = s_f32[:, 1088:1096]
    s_WihT = s_f32[:, 1096:5192]
    s_bgT = s_f32[:, 5192:5196]
    s_b16 = nc.alloc_sbuf_tensor("s_b16", [128, 4225], dt.bfloat16)
    s_WhT = s_b16[:, 0:4096]
    s_wout = s_b16[:, 4096:4097]
    s_I128 = s_b16[:, 4097:4225]

    s_hidT = nc.alloc_sbuf_tensor("s_hidT", [128, 512], dt.float32)
    # x_gates transposed: [128 gate-rows, gate-block m, batch] (hi/lo bf16)
    s_xT = nc.alloc_sbuf_tensor("s_xT", [128, 256], dt.float32)
    s_xres = nc.alloc_sbuf_tensor("s_xres", [128, 256], dt.float32)
    s_xThi = nc.alloc_sbuf_tensor("s_xThi", [128, 4, 64], dt.bfloat16)
    s_xTlo = nc.alloc_sbuf_tensor("s_xTlo", [128, 4, 64], dt.bfloat16)
    # per-stream recv buffers: 4-deep rotation, 8 slots x 32 batch cols
    recvA = [
        nc.alloc_sbuf_tensor(f"recvA{p}", [128, 256], dt.bfloat16) for p in range(NPH)
    ]
    recvB = [
        nc.alloc_sbuf_tensor(f"recvB{p}", [128, 256], dt.bfloat16) for p in range(NPH)
    ]
    gA = [nc.alloc_sbuf_tensor(f"gA{p}", [128, 128], dt.float32) for p in range(2)]
    gB = [nc.alloc_sbuf_tensor(f"gB{p}", [128, 128], dt.float32) for p in range(2)]
    thA = [nc.alloc_sbuf_tensor(f"thA{p}", [128, 32], dt.float32) for p in range(2)]
    thB = [nc.alloc_sbuf_tensor(f"thB{p}", [128, 32], dt.float32) for p in range(2)]
    # each send buffer padded to its own 512B-aligned footprint
    _hsA = [
        nc.alloc_sbuf_tensor(f"h_sendA{p}", [128, 256], dt.bfloat16) for p in range(2)
    ]
    _hsB = [
        nc.alloc_sbuf_tensor(f"h_sendB{p}", [128, 256], dt.bfloat16) for p in range(2)
    ]
    h_sendA = [t[:, 0:32] for t in _hsA]
    h_sendB = [t[:, 0:32] for t in _hsB]
    s_t1 = nc.alloc_sbuf_tensor("s_t1", [128, 32], dt.float32)
    s_t2 = nc.alloc_sbuf_tensor("s_t2", [128, 32], dt.float32)
    s_gt = nc.alloc_sbuf_tensor("s_gt", [128, 32], dt.float32)
    cA = [nc.alloc_sbuf_tensor(f"cA{p}", [128, 32], dt.float32) for p in range(2)]
    cB = [nc.alloc_sbuf_tensor(f"cB{p}", [128, 32], dt.float32) for p in range(2)]
    s_out = nc.alloc_sbuf_tensor("s_out", [64, Tn], dt.float32)

    p_hid = nc.alloc_psum_tensor("p_hid", [128, 512], dt.float32)
    p_x = nc.alloc_psum_tensor("p_x", [128, 512], dt.float32)
    p_gA = [
        nc.alloc_psum_tensor(f"p_gA{p}", [128, 512], dt.float32) for p in range(2)
    ]
    p_gB = [
        nc.alloc_psum_tensor(f"p_gB{p}", [128, 512], dt.float32) for p in range(2)
    ]
    p_out = nc.alloc_psum_tensor("p_out", [128, 512], dt.float32)
    p_fill = nc.alloc_psum_tensor("p_fill", [128, 512], dt.float32)

    s_srcA = [nc.alloc_semaphore(f"s_srcA{j}") for j in range(NC)]
    s_srcB = [nc.alloc_semaphore(f"s_srcB{j}") for j in range(NC)]
    s_peA = nc.alloc_semaphore("s_peA")
    s_peB = nc.alloc_semaphore("s_peB")
    s_sigA = nc.alloc_semaphore("s_sigA")
    s_sigB = nc.alloc_semaphore("s_sigB")
    s_thsA = nc.alloc_semaphore("s_thsA")
    s_thsB = nc.alloc_semaphore("s_thsB")
    s_cA = nc.alloc_semaphore("s_cA")
    s_cB = nc.alloc_semaphore("s_cB")
    s_hA = nc.alloc_semaphore("s_hA")
    s_hB = nc.alloc_semaphore("s_hB")
    s_locA = nc.alloc_semaphore("s_locA")
    s_locB = nc.alloc_semaphore("s_locB")
    s_prepA = nc.alloc_semaphore("s_prepA")
    s_prepB = nc.alloc_semaphore("s_prepB")
    s_ph = nc.alloc_semaphore("s_ph")
    s_xa = nc.alloc_semaphore("s_xa")
    s_v = nc.alloc_semaphore("s_v")
    s_xrdy = nc.alloc_semaphore("s_xrdy")
    s_osem = nc.alloc_semaphore("s_osem")
    s_fin = nc.alloc_semaphore("s_fin")
    dma_sem = nc.alloc_semaphore("dma_sem")

    SRC_INC = 16 if solo else 2

    # X-inject: pg[:, 0:128] = xT (hi then lo), via constant identity
    # stationary; the moving AP walks [4 gate blocks, 32 batch cols].
    # bslice selects the stream's batch half inside each 64-wide block.
    def x_inject(tensor, pg, bstart, final_stop=False):
        tensor.matmul(
            pg[:, 0:128],
            s_b16[:, 4097:4225],
            s_xThi[:, :, bstart : bstart + 32],
            start=True,
            stop=False,
        )
        mm = tensor.matmul(
            pg[:, 0:128],
            s_b16[:, 4097:4225],
            s_xTlo[:, :, bstart : bstart + 32],
            start=False,
            stop=final_stop,
        )
        return mm

    def fillers(tensor, n):
        for fi in range(n):
            tensor.matmul(
                p_fill[:, 0:FILLER_N],
                s_b16[:, 0:128],
                s_b16[:, 128 : 128 + FILLER_N],
                start=(fi == 0),
                stop=(fi == n - 1),
            )

    with nc.Block() as block:

        @block.sync
        def _(sync: bass.BassEngine):
            sync.dma_start(s_f32[:, :], d_f32[:, :]).then_inc(dma_sem, 16)
            sync.dma_start(s_b16[:, :], d_b16[:, :]).then_inc(dma_sem, 16)
            sync.wait_ge(s_fin, 1)
            sync.dma_start(d_out[:, :], s_out[:, :]).then_inc(dma_sem, 16)
            sync.wait_ge(dma_sem, 16 * (N_IN + 1))

        @block.tensor
        def _(tensor: bass.BassTensorEngine):
            tensor.wait_ge(dma_sem, 16 * N_IN)
            # phase 1a: hidden^T chunks = W_lin row-chunks @ latent^T
            for m in range(8):
                mm = tensor.matmul(
                    p_hid[:, 64 * m : 64 * m + 64],
                    s_f32[:, 64 + 128 * m : 64 + 128 * m + 128],
                    s_latT,
                    start=True,
                    stop=True,
                )
            mm.then_inc(s_ph, 1)  # s_ph = 1
            # HAM warmup BEFORE phase 1b so the fp32 1b matmuls run at
            # 2.4 GHz (also overlaps the ACT hidden-bias stage)
            for fi in range(12):
                tensor.matmul(
                    p_fill[:, 0:512],
                    s_b16[:, 0:128],
                    s_b16[:, 128:640],
                    start=(fi == 0),
                    stop=(fi == 11),
                )
            # phase 1b: x_gates TRANSPOSED: for each gate block m (f|g|i|o),
            # xT[:, 64m:64m+64] = Wih_m @ hidden^T, accumulated over 8
            # h-chunks k.  Stationary block (m,k) lives at col (m*8+k)*128.
            tensor.wait_ge(s_ph, 2)
            for m in range(4):
                for k in range(8):
                    mm = tensor.matmul(
                        p_x[:, 64 * m : 64 * m + 64],
                        s_f32[:, 1096 + (m * 8 + k) * 128 : 1096 + (m * 8 + k + 1) * 128],
                        s_hidT[:, 64 * k : 64 * k + 64],
                        start=(k == 0),
                        stop=(k == 7),
                    )
                mm.then_inc(s_ph, 1)  # s_ph = 3 + m
            # prologue: round-0 gates = X only
            tensor.wait_ge(s_xrdy, 1)
            x_inject(tensor, p_gA[0], 0, final_stop=True).then_inc(s_peA, 1)
            x_inject(tensor, p_gB[0], 32, final_stop=True).then_inc(s_peB, 1)

            for r in range(Tn):
                nfill_a = FILL_EARLY if r < 2 else FILL_A
                nfill_b = FILL_EARLY if r < 2 else FILL_B
                # ---- stream A ----
                if r >= 1:
                    par = r % NPH
                    pg = p_gA[r % 2]
                    for x in range(8):
                        tensor.wait_ge(s_srcA[x], SRC_INC * r)
                        for m in range(4):
                            mm = tensor.matmul(
                                pg[:, 32 * m : 32 * m + 32],
                                s_b16[:, (4 * x + m) * 128 : (4 * x + m + 1) * 128],
                                recvA[par][:, 32 * x : 32 * x + 32],
                                start=False,
                                stop=(x == 7 and m == 3),
                            )
                    mm.then_inc(s_peA, 1)  # r+1
                if r + 1 < Tn:
                    # X for round r+1 opens the pg[(r+1)%2] accumulation group
                    x_inject(tensor, p_gA[(r + 1) % 2], 0)
                if r >= 1:
                    tensor.wait_ge(s_hA, r)
                    tensor.matmul(
                        p_out[0:32, r - 1 : r],
                        h_sendA[r % 2],
                        s_b16[:, 4096:4097],
                        start=True,
                        stop=True,
                    )
                fillers(tensor, nfill_a)
                # ---- stream B ----
                if r >= 1:
                    par = r % NPH
                    pg = p_gB[r % 2]
                    for x in range(8):
                        tensor.wait_ge(s_srcB[x], SRC_INC * r)
                        for m in range(4):
                            mm = tensor.matmul(
                                pg[:, 32 * m : 32 * m + 32],
                                s_b16[:, (4 * x + m) * 128 : (4 * x + m + 1) * 128],
                                recvB[par][:, 32 * x : 32 * x + 32],
                                start=False,
                                stop=(x == 7 and m == 3),
                            )
                    mm.then_inc(s_peB, 1)  # r+1
                if r + 1 < Tn:
                    x_inject(tensor, p_gB[(r + 1) % 2], 32)
                if r >= 1:
                    tensor.wait_ge(s_hB, r)
                    tensor.matmul(
                        p_out[32:64, r - 1 : r],
                        h_sendB[r % 2],
                        s_b16[:, 4096:4097],
                        start=True,
                        stop=True,
                    )
                fillers(tensor, nfill_b)

            tensor.wait_ge(s_hA, Tn)
            tensor.matmul(
                p_out[0:32, Tn - 1 : Tn],
                h_sendA[Tn % 2],
                s_b16[:, 4096:4097],
                start=True,
                stop=True,
            ).then_inc(s_osem, 1)
            tensor.wait_ge(s_hB, Tn)
            tensor.matmul(
                p_out[32:64, Tn - 1 : Tn],
                h_sendB[Tn % 2],
                s_b16[:, 4096:4097],
                start=True,
                stop=True,
            ).then_inc(s_osem, 1)

        @block.scalar
        def _(scalar: bass.BassScalarEngine):
            scalar.wait_ge(s_ph, 1)
            for m in range(8):
                a = scalar.activation(
                    s_hidT[:, 64 * m : 64 * m + 64],
                    p_hid[:, 64 * m : 64 * m + 64],
                    AF.Identity,
                    bias=s_f32[:, 1088 + m : 1088 + m + 1],
                    scale=1.0,
                )
            a.then_inc(s_ph, 1)  # s_ph = 2
            # xT bias add per gate block (bias is per-partition here)
            # (wait for ALL four groups: reading a PSUM bank while the PE
            # still accumulates other columns of the same bank is unsafe)
            scalar.wait_ge(s_ph, 6)
            for m in range(4):
                scalar.activation(
                    s_xT[:, 64 * m : 64 * m + 64],
                    p_x[:, 64 * m : 64 * m + 64],
                    AF.Identity,
                    bias=s_f32[:, 5192 + m : 5192 + m + 1],
                    scale=1.0,
                ).then_inc(s_xa, 1)
            for r in range(Tn):
                scalar.wait_ge(s_peA, r + 1)
                scalar.activation(
                    gA[r % 2][:, 0:128], p_gA[r % 2][:, 0:128], AF.Sigmoid
                ).then_inc(s_sigA, 1)  # r+1
                scalar.wait_ge(s_cA, r + 1)
                scalar.activation(
                    thA[r % 2][:, :], cA[r % 2][:, :], AF.Tanh
                ).then_inc(s_thsA, 1)  # r+1
                scalar.wait_ge(s_peB, r + 1)
                scalar.activation(
                    gB[r % 2][:, 0:128], p_gB[r % 2][:, 0:128], AF.Sigmoid
                ).then_inc(s_sigB, 1)  # r+1
                scalar.wait_ge(s_cB, r + 1)
                scalar.activation(
                    thB[r % 2][:, :], cB[r % 2][:, :], AF.Tanh
                ).then_inc(s_thsB, 1)  # r+1
            scalar.wait_ge(s_osem, 2)
            scalar.activation(s_out[:, :], p_out[0:64, 0:Tn], AF.Copy).then_inc(
                s_fin, 1
            )

        @block.vector
        def _(vector: bass.BassVectorEngine):
            vector.wait_ge(s_xa, 4)
            vector.tensor_copy(s_xThi[:, :, :], s_xT[:, :]).then_inc(s_v, 1)
            vector.wait_ge(s_v, 1)
            vector.tensor_tensor(
                s_xres[:, :], s_xT[:, :], s_xThi[:, :, :], ALU.subtract
            ).then_inc(s_v, 1)
            vector.wait_ge(s_v, 2)
            vector.tensor_copy(s_xTlo[:, :, :], s_xres[:, :])
            vector.memset(cA[1][:, :], 0.0)
            vector.memset(cB[1][:, :], 0.0).then_inc(s_xrdy, 1)
            # intra-DVE RAW edges (gt->t2, t2->c) carry explicit self-sems:
            # back-to-back DVE ops can read an operand before the prior op's
            # write fully lands.
            for r in range(Tn):
                # ---- stream A cell ----
                g = gA[r % 2]
                if r >= 2 and not solo:
                    vector.wait_ge(s_locA, 16 * (r - 1))
                vector.wait_ge(s_sigA, r + 1)
                vector.tensor_tensor(
                    s_t1[:, :], g[:, SG_F], cA[(r + 1) % 2][:, :], ALU.mult
                ).then_inc(s_v, 1)  # 6r+3
                vector.scalar_tensor_tensor(
                    s_t2[:, :], g[:, SG_G], -0.5, g[:, SG_I], ALU.add, ALU.mult
                ).then_inc(s_v, 2)  # 6r+5  (= t2/2)
                vector.wait_ge(s_v, 6 * r + 5)
                vector.scalar_tensor_tensor(
                    cA[r % 2][:, :], s_t2[:, :], 2.0, s_t1[:, :], ALU.mult, ALU.add
                ).then_inc(s_cA, 1)  # r+1
                vector.wait_ge(s_thsA, r + 1)
                vector.tensor_tensor(
                    h_sendA[(r + 1) % 2], g[:, SG_O], thA[r % 2][:, :], ALU.mult
                ).then_inc(s_hA, 1)  # r+1
                # ---- stream B cell ----
                g = gB[r % 2]
                if r >= 2 and not solo:
                    vector.wait_ge(s_locB, 16 * (r - 1))
                vector.wait_ge(s_sigB, r + 1)
                vector.tensor_tensor(
                    s_t1[:, :], g[:, SG_F], cB[(r + 1) % 2][:, :], ALU.mult
                ).then_inc(s_v, 1)  # 6r+6
                vector.scalar_tensor_tensor(
                    s_t2[:, :], g[:, SG_G], -0.5, g[:, SG_I], ALU.add, ALU.mult
                ).then_inc(s_v, 2)  # 6r+8  (= t2/2)
                vector.wait_ge(s_v, 6 * r + 8)
                vector.scalar_tensor_tensor(
                    cB[r % 2][:, :], s_t2[:, :], 2.0, s_t1[:, :], ALU.mult, ALU.add
                ).then_inc(s_cB, 1)  # r+1
                vector.wait_ge(s_thsB, r + 1)
                vector.tensor_tensor(
                    h_sendB[(r + 1) % 2], g[:, SG_O], thB[r % 2][:, :], ALU.mult
                ).then_inc(s_hB, 1)  # r+1

        @block.gpsimd
        def _(gpsimd: bass.BassGpSimd):
            if solo:
                for r in range(Tn):
                    gpsimd.wait_ge(s_hA, r + 1)
                    for j in range(8):
                        gpsimd.dma_start(
                            recvA[(r + 1) % NPH][:, 32 * j : 32 * j + 32],
                            h_sendA[(r + 1) % 2],
                        ).then_inc(s_srcA[j], 16)
                    gpsimd.wait_ge(s_hB, r + 1)
                    for j in range(8):
                        gpsimd.dma_start(
                            recvB[(r + 1) % NPH][:, 32 * j : 32 * j + 32],
                            h_sendB[(r + 1) % 2],
                        ).then_inc(s_srcB[j], 16)
                return
            gpsimd.bir_kernel_barrier_wait([list(range(NC))])
            pid = gpsimd.partition_id()
            for case in gpsimd.Switch(pid, NC):
                rdests = [(0, j) for j in range(NC)]
                # prologue: prep round-0 frames (A then B, strict FIFO order)
                gpsimd.remote_dma_broadcast(
                    out_ap=recvA[1][:, 32 * case : 32 * case + 32],
                    in_ap=h_sendA[1],
                    remote_sem=s_srcA[case],
                    local_sem=s_locA,
                    rdests=rdests,
                ).then_inc(s_prepA, 1)
                gpsimd.remote_dma_broadcast(
                    out_ap=recvB[1][:, 32 * case : 32 * case + 32],
                    in_ap=h_sendB[1],
                    remote_sem=s_srcB[case],
                    local_sem=s_locB,
                    rdests=rdests,
                ).then_inc(s_prepB, 1)
                for r in range(Tn):
                    # prep(r+2) frames are emitted in the A,B ring order but
                    # BETWEEN the two triggers, so descriptor generation never
                    # delays a trigger whose h just became ready.
                    gpsimd.wait_ge(s_prepA, r + 1)
                    gpsimd.wait_ge(s_hA, r + 1)
                    gpsimd.trigger_dma(count=1)  # fires frame A(r)
                    if r + 1 < Tn:
                        gpsimd.remote_dma_broadcast(
                            out_ap=recvA[(r + 2) % NPH][
                                :, 32 * case : 32 * case + 32
                            ],
                            in_ap=h_sendA[(r + 2) % 2],
                            remote_sem=s_srcA[case],
                            local_sem=s_locA,
                            rdests=rdests,
                        ).then_inc(s_prepA, 1)
                    gpsimd.wait_ge(s_prepB, r + 1)
                    gpsimd.wait_ge(s_hB, r + 1)
                    gpsimd.trigger_dma(count=1)  # fires frame B(r)
                    if r + 1 < Tn:
                        gpsimd.remote_dma_broadcast(
                            out_ap=recvB[(r + 2) % NPH][
                                :, 32 * case : 32 * case + 32
                            ],
                            in_ap=h_sendB[(r + 2) % 2],
                            remote_sem=s_srcB[case],
                            local_sem=s_locB,
                            rdests=rdests,
                        ).then_inc(s_prepB, 1)
                    gpsimd.wait_ge(s_locA, 16 * r)
                    gpsimd.wait_ge(s_locB, 16 * r)

    nc.has_collectives = not solo
    nc.finalize()
    return nc


def _prep_core_inputs(inputs: dict, r: int, src_row=None) -> dict:
    if src_row is None:
        src_row = list(range(8))  # slot j holds logical core j's H-chunk
    f32 = np.float32
    bf16 = ml_dtypes.bfloat16
    latent = np.asarray(inputs["latent"], f32)
    W_lin = np.asarray(inputs["W_lin"], f32)
    b_lin = np.asarray(inputs["b_lin"], f32)
    W_ih = np.asarray(inputs["W_ih"], f32)
    W_hh = np.asarray(inputs["W_hh"], f32)
    b_ih = np.asarray(inputs["b_ih"], f32)
    b_hh = np.asarray(inputs["b_hh"], f32)
    W_out = np.asarray(inputs["W_out"], f32)

    HS = 128
    sl = slice(HS * r, HS * (r + 1))

    # g-gate (pytorch index 2) rows scaled by 2: tanh(x) = 2*sigmoid(2x)-1,
    # so the device applies one sigmoid to all four gates and DVE fixes g up.
    gscale = {2: 2.0}

    # WihT: stationary block (m, k) at cols (m*8+k)*128: Wih[gate block m,
    # h-chunk k]^T so the device can compute x_gates transposed.
    WihT = np.zeros((128, 4096), f32)
    for m, g in enumerate(GATE_ORDER):
        blk_rows = gscale.get(g, 1.0) * W_ih[g * H + HS * r : g * H + HS * (r + 1), :]
        for k in range(8):
            WihT[:, (m * 8 + k) * 128 : (m * 8 + k + 1) * 128] = blk_rows[
                :, HS * k : HS * (k + 1)
            ].T

    bgv = b_ih + b_hh
    bgT = np.stack(
        [
            gscale.get(g, 1.0) * bgv[g * H + HS * r : g * H + HS * (r + 1)]
            for g in GATE_ORDER
        ],
        axis=1,
    )  # [128, 4]

    WhT = np.zeros((128, 4096), f32)
    for x in range(8):
        srcc = src_row[x]
        for m, g in enumerate(GATE_ORDER):
            blk = W_hh[
                g * H + HS * r : g * H + HS * (r + 1), HS * srcc : HS * (srcc + 1)
            ]
            WhT[:, (4 * x + m) * 128 : (4 * x + m + 1) * 128] = gscale.get(
                g, 1.0
            ) * blk.T

    f32blob = np.concatenate(
        [
            np.ascontiguousarray(latent.T),
            np.ascontiguousarray(W_lin.T),
            np.ascontiguousarray(b_lin.reshape(8, 128).T),
            WihT,
            np.ascontiguousarray(bgT),
        ],
        axis=1,
    )
    b16blob = np.concatenate(
        [
            WhT.astype(bf16),
            np.ascontiguousarray(W_out[0, sl].reshape(128, 1)).astype(bf16),
            np.eye(128, dtype=f32).astype(bf16),
        ],
        axis=1,
    )
    return {"f32blob": f32blob, "b16blob": b16blob}


def _run(inputs: dict, trace: bool = False):
    from concourse.bass_utils import run_bass_kernel_spmd

    if "nc" not in _cache:
        _cache["nc"] = _build_lstm_nc(T)
    nc = _cache["nc"]
    in_maps = [_prep_core_inputs(inputs, r) for r in range(NC)]
    res = run_bass_kernel_spmd(
        nc, in_maps, core_ids=list(range(NC)), trace=trace
    )
    outs = [np.asarray(res.results[r]["outp"], np.float64) for r in range(NC)]
    b_out = np.asarray(inputs["b_out"], np.float64)
    total = outs[0]
    for o in outs[1:]:
        total = total + o
    total = total + b_out[0]
    out = total[:, :, None].astype(np.float32)
    return out, res


def kernel(**inputs) -> np.ndarray:
    seq_len = int(inputs.get("seq_len", T))
    assert seq_len == T, f"kernel hardcoded for seq_len={T}, got {seq_len}"
    out, _ = _run(inputs, trace=False)
    return out


# revision 16
# speedup vs baseline: 1.1224x; 1.1224x over previous
"""nn_Decoder (LSTM decoder) Trainium2 Bass kernel, 8-core tensor-parallel,
two phase-shifted batch streams.

Strategy (hardcoded for B=64, L=128, H=1024, O=1, T=256, 8 cores):
  The 4H=4096 gate rows are sharded 8 ways: each core owns a 128-row H-slice
  of each gate (layout f|g|i|o), computes gates transposed on PE (W_hh^T
  blocks stationary in bf16, h^T streamed), does the cell elementwise on
  ACT/DVE, and broadcasts its h^T chunk to all peers each step via one
  8-destination remote_dma_broadcast.

  The batch is split into two independent 32-wide streams (A = batch 0:32,
  B = 32:64) running the same recurrence phase-shifted by ~half a step.
  While stream A's h-broadcast is in flight, the engines process stream B's
  matmuls / cell, and vice versa.  Both streams' broadcast frames share
  SWDGE queue 0 in strict A,B alternation (a second SWDGE queue silently
  corrupts transfers on this runtime).

  v2 changes vs the 1.60 ms baseline (trace-driven):
  - The baseline ran the PE at K=4/8 HAM throttle (1.2 GHz) for 97% of the
    kernel: one >3.4us PE-idle window during the round-0/1 pipeline fill
    re-throttled it, and the steady state never has a 3.4us contiguous
    busy stretch to re-warm.  Rounds 0-1 now run big filler sections
    (FILL_EARLY) so the PE stays continuously busy through the pipeline
    fill and keeps the 2.4 GHz clock it earned during phase-1 matmuls.
  - x_gates is computed TRANSPOSED in phase 1b ([128 gate-rows, 64 batch]
    per gate block) and re-injected each round with a single N=128 matmul
    per (stream, hi/lo) using a constant identity stationary: 4 LDW+MM
    pairs per round instead of 16.
  - One [128,128] sigmoid per stream per round instead of 2x [128,64]
    (ACT ops are ~250ns fixed overhead + ~1 col/cycle).

  Other tricks carried over from the baseline:
  - Per-SOURCE arrival semaphores: each receiver's PE consumes h chunks as
    they arrive instead of waiting for all 8.
  - g-gate tanh folded into the gate sigmoid (host scales g rows by 2,
    DVE fixes up with *2-1).
  - Cell state + temporaries in SBUF.
  - x_gates re-injected into PSUM via identity matmuls (bf16 hi+lo split).
"""

import numpy as np
import ml_dtypes

B, L, H, O, T = 64, 128, 1024, 1, 256
NC = 8
NPH = 4
# device gate-column order f|g|i|o (indices into pytorch's i,f,g,o row blocks)
GATE_ORDER = [1, 2, 0, 3]
# per-stream gate slices in the [128, 128] gates tile (4 gates x 32 batch)
SG_F = slice(0, 32)
SG_G = slice(32, 64)
SG_I = slice(64, 96)
SG_O = slice(96, 128)
# Filler matmuls per stream section: keep the PE the (slightly) binding
# resource so rounds run PE-paced in lockstep across cores, and never let a
# PE-idle window reach the ~3.4us HAM MID threshold.
FILLER_N = 256
FILL_A = 2
FILL_B = 2
# Rounds 0-1 (pipeline fill) get large filler sections: the first h
# broadcasts take a full serial chain, and a single >3.4us PE-idle window
# there re-throttles the PE to 1.2 GHz for the whole kernel.
FILL_EARLY = 6

_cache = {}


def _build_lstm_nc(T_steps=T, solo=False, detect_races=True):
    import concourse.bacc as bacc
    import concourse.bass as bass
    import concourse.mybir as mybir

    dt = mybir.dt
    AF = mybir.ActivationFunctionType
    ALU = mybir.AluOpType
    Tn = T_steps

    nc = bacc.Bacc(
        None,
        target_bir_lowering=False,
        debug=False,
        num_devices=NC,
        detect_race_conditions=detect_races,
    )

    # all fp32 inputs packed as one [128, 5196] blob: latT|WlinT|blinT|WihT|bgT
    d_f32 = nc.dram_tensor("f32blob", [128, 5196], dt.float32, kind="ExternalInput")
    # all bf16 inputs packed as one [128, 4225] blob: WhT|wout|I128
    d_b16 = nc.dram_tensor("b16blob", [128, 4225], dt.bfloat16, kind="ExternalInput")
    d_out = nc.dram_tensor("outp", [64, Tn], dt.float32, kind="ExternalOutput")
    N_IN = 2

    s_f32 = nc.alloc_sbuf_tensor("s_f32", [128, 5196], dt.float32)
    s_latT = s_f32[:, 0:64]
    s_WlinT = s_f32[:, 64:1088]
    s_blinT = s_f32[:, 1088:1096]
    s_WihT = s_f32[:, 1096:5192]
    s_bgT = s_f32[:, 5192:5196]
    s_b16 = nc.alloc_sbuf_tensor("s_b16", [128, 4225], dt.bfloat16)
    s_WhT = s_b16[:, 0:4096]
    s_wout = s_b16[:, 4096:4097]
    s_I128 = s_b16[:, 4097:4225]

    s_hidT = nc.alloc_sbuf_tensor("s_hidT", [128, 512], dt.float32)
    # x_gates transposed: [128 gate-rows, gate-block m, batch] (hi/lo bf16)
    s_xT = nc.alloc_sbuf_tensor("s_xT", [128, 256], dt.float32)
    s_xres = nc.alloc_sbuf_tensor("s_xres", [128, 256], dt.float32)
    s_xThi = nc.alloc_sbuf_tensor("s_xThi", [128, 4, 64], dt.bfloat16)
    s_xTlo = nc.alloc_sbuf_tensor("s_xTlo", [128, 4, 64], dt.bfloat16)
    # per-stream recv buffers: 4-deep rotation, 8 slots x 32 batch cols
    recvA = [
        nc.alloc_sbuf_tensor(f"recvA{p}", [128, 256], dt.bfloat16) for p in range(NPH)
    ]
    recvB = [
        nc.alloc_sbuf_tensor(f"recvB{p}", [128, 256], dt.bfloat16) for p in range(NPH)
    ]
    gA = [nc.alloc_sbuf_tensor(f"gA{p}", [128, 128], dt.float32) for p in range(2)]
    gB = [nc.alloc_sbuf_tensor(f"gB{p}", [128, 128], dt.float32) for p in range(2)]
    thA = [nc.alloc_sbuf_tensor(f"thA{p}", [128, 32], dt.float32) for p in range(2)]
    thB = [nc.alloc_sbuf_tensor(f"thB{p}", [128, 32], dt.float32) for p in range(2)]
    # each send buffer padded to its own 512B-aligned footprint
    _hsA = [
        nc.alloc_sbuf_tensor(f"h_sendA{p}", [128, 256], dt.bfloat16) for p in range(2)
    ]
    _hsB = [
        nc.alloc_sbuf_tensor(f"h_sendB{p}", [128, 256], dt.bfloat16) for p in range(2)
    ]
    h_sendA = [t[:, 0:32] for t in _hsA]
    h_sendB = [t[:, 0:32] for t in _hsB]
    s_t1 = nc.alloc_sbuf_tensor("s_t1", [128, 32], dt.float32)
    s_t2 = nc.alloc_sbuf_tensor("s_t2", [128, 32], dt.float32)
    s_gt = nc.alloc_sbuf_tensor("s_gt", [128, 32], dt.float32)
    cA = [nc.alloc_sbuf_tensor(f"cA{p}", [128, 32], dt.float32) for p in range(2)]
    cB = [nc.alloc_sbuf_tensor(f"cB{p}", [128, 32], dt.float32) for p in range(2)]
    s_out = nc.alloc_sbuf_tensor("s_out", [64, Tn], dt.float32)

    p_hid = nc.alloc_psum_tensor("p_hid", [128, 512], dt.float32)
    p_x = nc.alloc_psum_tensor("p_x", [128, 512], dt.float32)
    p_gA = [
        nc.alloc_psum_tensor(f"p_gA{p}", [128, 512], dt.float32) for p in range(2)
    ]
    p_gB = [
        nc.alloc_psum_tensor(f"p_gB{p}", [128, 512], dt.float32) for p in range(2)
    ]
    p_out = nc.alloc_psum_tensor("p_out", [128, 512], dt.float32)
    p_fill = nc.alloc_psum_tensor("p_fill", [128, 512], dt.float32)

    s_srcA = [nc.alloc_semaphore(f"s_srcA{j}") for j in range(NC)]
    s_srcB = [nc.alloc_semaphore(f"s_srcB{j}") for j in range(NC)]
    s_peA = nc.alloc_semaphore("s_peA")
    s_peB = nc.alloc_semaphore("s_peB")
    s_sigA = nc.alloc_semaphore("s_sigA")
    s_sigB = nc.alloc_semaphore("s_sigB")
    s_thsA = nc.alloc_semaphore("s_thsA")
    s_thsB = nc.alloc_semaphore("s_thsB")
    s_cA = nc.alloc_semaphore("s_cA")
    s_cB = nc.alloc_semaphore("s_cB")
    s_hA = nc.alloc_semaphore("s_hA")
    s_hB = nc.alloc_semaphore("s_hB")
    s_locA = nc.alloc_semaphore("s_locA")
    s_locB = nc.alloc_semaphore("s_locB")
    s_prepA = nc.alloc_semaphore("s_prepA")
    s_prepB = nc.alloc_semaphore("s_prepB")
    s_ph = nc.alloc_semaphore("s_ph")
    s_xa = nc.alloc_semaphore("s_xa")
    s_v = nc.alloc_semaphore("s_v")
    s_xrdy = nc.alloc_semaphore("s_xrdy")
    s_osem = nc.alloc_semaphore("s_osem")
    s_fin = nc.alloc_semaphore("s_fin")
    dma_sem = nc.alloc_semaphore("dma_sem")

    SRC_INC = 16 if solo else 2

    # X-inject: pg[:, 0:128] = xT (hi then lo), via constant identity
    # stationary; the moving AP walks [4 gate blocks, 32 batch cols].
    # bslice selects the stream's batch half inside each 64-wide block.
    def x_inject(tensor, pg, bstart, final_stop=False):
        tensor.matmul(
            pg[:, 0:128],
            s_b16[:, 4097:4225],
            s_xThi[:, :, bstart : bstart + 32],
            start=True,
            stop=False,
        )
        mm = tensor.matmul(
            pg[:, 0:128],
            s_b16[:, 4097:4225],
            s_xTlo[:, :, bstart : bstart + 32],
            start=False,
            stop=final_stop,
        )
        return mm

    def fillers(tensor, n):
        for fi in range(n):
            tensor.matmul(
                p_fill[:, 0:FILLER_N],
                s_b16[:, 0:128],
                s_b16[:, 128 : 128 + FILLER_N],
                start=(fi == 0),
                stop=(fi == n - 1),
            )

    with nc.Block() as block:

        @block.sync
        def _(sync: bass.BassEngine):
            sync.dma_start(s_f32[:, :], d_f32[:, :]).then_inc(dma_sem, 16)
            sync.dma_start(s_b16[:, :], d_b16[:, :]).then_inc(dma_sem, 16)
            sync.wait_ge(s_fin, 1)
            sync.dma_start(d_out[:, :], s_out[:, :]).then_inc(dma_sem, 16)
            sync.wait_ge(dma_sem, 16 * (N_IN + 1))

        @block.tensor
        def _(tensor: bass.BassTensorEngine):
            tensor.wait_ge(dma_sem, 16 * N_IN)
            # phase 1a: hidden^T chunks = W_lin row-chunks @ latent^T
            for m in range(8):
                mm = tensor.matmul(
                    p_hid[:, 64 * m : 64 * m + 64],
                    s_f32[:, 64 + 128 * m : 64 + 128 * m + 128],
                    s_latT,
                    start=True,
                    stop=True,
                )
            mm.then_inc(s_ph, 1)  # s_ph = 1
            # HAM warmup BEFORE phase 1b so the fp32 1b matmuls run at
            # 2.4 GHz (also overlaps the ACT hidden-bias stage)
            for fi in range(12):
                tensor.matmul(
                    p_fill[:, 0:512],
                    s_b16[:, 0:128],
                    s_b16[:, 128:640],
                    start=(fi == 0),
                    stop=(fi == 11),
                )
            # phase 1b: x_gates TRANSPOSED: for each gate block m (f|g|i|o),
            # xT[:, 64m:64m+64] = Wih_m @ hidden^T, accumulated over 8
            # h-chunks k.  Stationary block (m,k) lives at col (m*8+k)*128.
            tensor.wait_ge(s_ph, 2)
            for m in range(4):
                for k in range(8):
                    mm = tensor.matmul(
                        p_x[:, 64 * m : 64 * m + 64],
                        s_f32[:, 1096 + (m * 8 + k) * 128 : 1096 + (m * 8 + k + 1) * 128],
                        s_hidT[:, 64 * k : 64 * k + 64],
                        start=(k == 0),
                        stop=(k == 7),
                    )
                mm.then_inc(s_ph, 1)  # s_ph = 3 + m
            # prologue: round-0 gates = X only
            tensor.wait_ge(s_xrdy, 1)
            x_inject(tensor, p_gA[0], 0, final_stop=True).then_inc(s_peA, 1)
            x_inject(tensor, p_gB[0], 32, final_stop=True).then_inc(s_peB, 1)

            for r in range(Tn):
                nfill_a = FILL_EARLY if r < 2 else FILL_A
                nfill_b = FILL_EARLY if r < 2 else FILL_B
                # ---- stream A ----
                if r >= 1:
                    par = r % NPH
                    pg = p_gA[r % 2]
                    for x in range(8):
                        tensor.wait_ge(s_srcA[x], SRC_INC * r)
                        for m in range(4):
                            mm = tensor.matmul(
                                pg[:, 32 * m : 32 * m + 32],
                                s_b16[:, (4 * x + m) * 128 : (4 * x + m + 1) * 128],
                                recvA[par][:, 32 * x : 32 * x + 32],
                                start=False,
                                stop=(x == 7 and m == 3),
                            )
                    mm.then_inc(s_peA, 1)  # r+1
                if r + 1 < Tn:
                    # X for round r+1 opens the pg[(r+1)%2] accumulation group
                    x_inject(tensor, p_gA[(r + 1) % 2], 0)
                if r >= 1:
                    tensor.wait_ge(s_hA, r)
                    tensor.matmul(
                        p_out[0:32, r - 1 : r],
                        h_sendA[r % 2],
                        s_b16[:, 4096:4097],
                        start=True,
                        stop=True,
                    )
                fillers(tensor, nfill_a)
                # ---- stream B ----
                if r >= 1:
                    par = r % NPH
                    pg = p_gB[r % 2]
                    for x in range(8):
                        tensor.wait_ge(s_srcB[x], SRC_INC * r)
                        for m in range(4):
                            mm = tensor.matmul(
                                pg[:, 32 * m : 32 * m + 32],
                                s_b16[:, (4 * x + m) * 128 : (4 * x + m + 1) * 128],
                                recvB[par][:, 32 * x : 32 * x + 32],
                                start=False,
                                stop=(x == 7 and m == 3),
                            )
                    mm.then_inc(s_peB, 1)  # r+1
                if r + 1 < Tn:
                    x_inject(tensor, p_gB[(r + 1) % 2], 32)
                if r >= 1:
                    tensor.wait_ge(s_hB, r)
                    tensor.matmul(
                        p_out[32:64, r - 1 : r],
                        h_sendB[r % 2],
                        s_b16[:, 4096:4097],
                        start=True,
                        stop=True,
                    )
                fillers(tensor, nfill_b)

            tensor.wait_ge(s_hA, Tn)
            tensor.matmul(
                p_out[0:32, Tn - 1 : Tn],
                h_sendA[Tn % 2],
                s_b16[:, 4096:4097],
                start=True,
                stop=True,
            ).then_inc(s_osem, 1)
            tensor.wait_ge(s_hB, Tn)
            tensor.matmul(
                p_out[32:64, Tn - 1 : Tn],
                h_sendB[Tn % 2],
                s_b16[:, 4096:4097],
                start=True,
                stop=True,
            ).then_inc(s_osem, 1)

        @block.scalar
        def _(scalar: bass.BassScalarEngine):
            scalar.wait_ge(s_ph, 1)
            for m in range(8):
                a = scalar.activation(
                    s_hidT[:, 64 * m : 64 * m + 64],
                    p_hid[:, 64 * m : 64 * m + 64],
                    AF.Identity,
                    bias=s_f32[:, 1088 + m : 1088 + m + 1],
                    scale=1.0,
                )
            a.then_inc(s_ph, 1)  # s_ph = 2
            # xT bias add per gate block (bias is per-partition here)
            # (wait for ALL four groups: reading a PSUM bank while the PE
            # still accumulates other columns of the same bank is unsafe)
            scalar.wait_ge(s_ph, 6)
            for m in range(4):
                scalar.activation(
                    s_xT[:, 64 * m : 64 * m + 64],
                    p_x[:, 64 * m : 64 * m + 64],
                    AF.Identity,
                    bias=s_f32[:, 5192 + m : 5192 + m + 1],
                    scale=1.0,
                ).then_inc(s_xa, 1)
            for r in range(Tn):
                scalar.wait_ge(s_peA, r + 1)
                scalar.activation(
                    gA[r % 2][:, 0:128], p_gA[r % 2][:, 0:128], AF.Sigmoid
                ).then_inc(s_sigA, 1)  # r+1
                scalar.wait_ge(s_cA, r + 1)
                scalar.activation(
                    thA[r % 2][:, :], cA[r % 2][:, :], AF.Tanh
                ).then_inc(s_thsA, 1)  # r+1
                scalar.wait_ge(s_peB, r + 1)
                scalar.activation(
                    gB[r % 2][:, 0:128], p_gB[r % 2][:, 0:128], AF.Sigmoid
                ).then_inc(s_sigB, 1)  # r+1
                scalar.wait_ge(s_cB, r + 1)
                scalar.activation(
                    thB[r % 2][:, :], cB[r % 2][:, :], AF.Tanh
                ).then_inc(s_thsB, 1)  # r+1
            scalar.wait_ge(s_osem, 2)
            scalar.activation(s_out[:, :], p_out[0:64, 0:Tn], AF.Copy).then_inc(
                s_fin, 1
            )

        @block.vector
        def _(vector: bass.BassVectorEngine):
            vector.wait_ge(s_xa, 4)
            vector.tensor_copy(s_xThi[:, :, :], s_xT[:, :]).then_inc(s_v, 1)
            vector.wait_ge(s_v, 1)
            vector.tensor_tensor(
                s_xres[:, :], s_xT[:, :], s_xThi[:, :, :], ALU.subtract
            ).then_inc(s_v, 1)
            vector.wait_ge(s_v, 2)
            vector.tensor_copy(s_xTlo[:, :, :], s_xres[:, :])
            vector.memset(cA[1][:, :], 0.0)
            vector.memset(cB[1][:, :], 0.0).then_inc(s_xrdy, 1)
            # intra-DVE RAW edges (gt->t2, t2->c) carry explicit self-sems:
            # back-to-back DVE ops can read an operand before the prior op's
            # write fully lands.
            for r in range(Tn):
                # ---- stream A cell ----
                g = gA[r % 2]
                vector.wait_ge(s_sigA, r + 1)
                vector.tensor_tensor(
                    s_t1[:, :], g[:, SG_F], cA[(r + 1) % 2][:, :], ALU.mult
                ).then_inc(s_v, 1)  # 6r+3
                vector.scalar_tensor_tensor(
                    s_t2[:, :], g[:, SG_G], -0.5, g[:, SG_I], ALU.add, ALU.mult
                ).then_inc(s_v, 2)  # 6r+5  (= t2/2)
                vector.wait_ge(s_v, 6 * r + 5)
                vector.scalar_tensor_tensor(
                    cA[r % 2][:, :], s_t2[:, :], 2.0, s_t1[:, :], ALU.mult, ALU.add
                ).then_inc(s_cA, 1)  # r+1
                vector.wait_ge(s_thsA, r + 1)
                if r >= 2 and not solo:
                    vector.wait_ge(s_locA, 16 * (r - 1))
                vector.tensor_tensor(
                    h_sendA[(r + 1) % 2], g[:, SG_O], thA[r % 2][:, :], ALU.mult
                ).then_inc(s_hA, 1)  # r+1
                # ---- stream B cell ----
                g = gB[r % 2]
                vector.wait_ge(s_sigB, r + 1)
                vector.tensor_tensor(
                    s_t1[:, :], g[:, SG_F], cB[(r + 1) % 2][:, :], ALU.mult
                ).then_inc(s_v, 1)  # 6r+6
                vector.scalar_tensor_tensor(
                    s_t2[:, :], g[:, SG_G], -0.5, g[:, SG_I], ALU.add, ALU.mult
                ).then_inc(s_v, 2)  # 6r+8  (= t2/2)
                vector.wait_ge(s_v, 6 * r + 8)
                vector.scalar_tensor_tensor(
                    cB[r % 2][:, :], s_t2[:, :], 2.0, s_t1[:, :], ALU.mult, ALU.add
                ).then_inc(s_cB, 1)  # r+1
                vector.wait_ge(s_thsB, r + 1)
                if r >= 2 and not solo:
                    vector.wait_ge(s_locB, 16 * (r - 1))
                vector.tensor_tensor(
                    h_sendB[(r + 1) % 2], g[:, SG_O], thB[r % 2][:, :], ALU.mult
                ).then_inc(s_hB, 1)  # r+1

        @block.gpsimd
        def _(gpsimd: bass.BassGpSimd):
            if solo:
                for r in range(Tn):
                    gpsimd.wait_ge(s_hA, r + 1)
                    for j in range(8):
                        gpsimd.dma_start(
                            recvA[(r + 1) % NPH][:, 32 * j : 32 * j + 32],
                            h_sendA[(r + 1) % 2],
                        ).then_inc(s_srcA[j], 16)
                    gpsimd.wait_ge(s_hB, r + 1)
                    for j in range(8):
                        gpsimd.dma_start(
                            recvB[(r + 1) % NPH][:, 32 * j : 32 * j + 32],
                            h_sendB[(r + 1) % 2],
                        ).then_inc(s_srcB[j], 16)
                return
            gpsimd.bir_kernel_barrier_wait([list(range(NC))])
            pid = gpsimd.partition_id()
            for case in gpsimd.Switch(pid, NC):
                rdests = [(0, j) for j in range(NC)]
                # prologue: prep round-0 frames (A then B, strict FIFO order)
                gpsimd.remote_dma_broadcast(
                    out_ap=recvA[1][:, 32 * case : 32 * case + 32],
                    in_ap=h_sendA[1],
                    remote_sem=s_srcA[case],
                    local_sem=s_locA,
                    rdests=rdests,
                ).then_inc(s_prepA, 1)
                gpsimd.remote_dma_broadcast(
                    out_ap=recvB[1][:, 32 * case : 32 * case + 32],
                    in_ap=h_sendB[1],
                    remote_sem=s_srcB[case],
                    local_sem=s_locB,
                    rdests=rdests,
                ).then_inc(s_prepB, 1)
                for r in range(Tn):
                    # prep(r+2) frames are emitted in the A,B ring order but
                    # BETWEEN the two triggers, so descriptor generation never
                    # delays a trigger whose h just became ready.
                    gpsimd.wait_ge(s_prepA, r + 1)
                    gpsimd.wait_ge(s_hA, r + 1)
                    gpsimd.trigger_dma(count=1)  # fires frame A(r)
                    if r + 1 < Tn:
                        gpsimd.remote_dma_broadcast(
                            out_ap=recvA[(r + 2) % NPH][
                                :, 32 * case : 32 * case + 32
                            ],
                            in_ap=h_sendA[(r + 2) % 2],
                            remote_sem=s_srcA[case],
                            local_sem=s_locA,
                            rdests=rdests,
                        ).then_inc(s_prepA, 1)
                    gpsimd.wait_ge(s_prepB, r + 1)
                    gpsimd.wait_ge(s_hB, r + 1)
                    gpsimd.trigger_dma(count=1)  # fires frame B(r)
                    if r + 1 < Tn:
                        gpsimd.remote_dma_broadcast(
                            out_ap=recvB[(r + 2) % NPH][
                                :, 32 * case : 32 * case + 32
                            ],
                            in_ap=h_sendB[(r + 2) % 2],
                            remote_sem=s_srcB[case],
                            local_sem=s_locB,
                            rdests=rdests,
                        ).then_inc(s_prepB, 1)
                    gpsimd.wait_ge(s_locA, 16 * r)
                    gpsimd.wait_ge(s_locB, 16 * r)

    nc.has_collectives = not solo
    nc.finalize()
    return nc


def _prep_core_inputs(inputs: dict, r: int, src_row=None) -> dict:
    if src_row is None:
        src_row = list(range(8))  # slot j holds logical core j's H-chunk
    f32 = np.float32
    bf16 = ml_dtypes.bfloat16
    latent = np.asarray(inputs["latent"], f32)
    W_lin = np.asarray(inputs["W_lin"], f32)
    b_lin = np.asarray(inputs["b_lin"], f32)
    W_ih = np.asarray(inputs["W_ih"], f32)
    W_hh = np.asarray(inputs["W_hh"], f32)
    b_ih = np.asarray(inputs["b_ih"], f32)
    b_hh = np.asarray(inputs["b_hh"], f32)
    W_out = np.asarray(inputs["W_out"], f32)

    HS = 128
    sl = slice(HS * r, HS * (r + 1))

    # g-gate (pytorch index 2) rows scaled by 2: tanh(x) = 2*sigmoid(2x)-1,
    # so the device applies one sigmoid to all four gates and DVE fixes g up.
    gscale = {2: 2.0}

    # WihT: stationary block (m, k) at cols (m*8+k)*128: Wih[gate block m,
    # h-chunk k]^T so the device can compute x_gates transposed.
    WihT = np.zeros((128, 4096), f32)
    for m, g in enumerate(GATE_ORDER):
        blk_rows = gscale.get(g, 1.0) * W_ih[g * H + HS * r : g * H + HS * (r + 1), :]
        for k in range(8):
            WihT[:, (m * 8 + k) * 128 : (m * 8 + k + 1) * 128] = blk_rows[
                :, HS * k : HS * (k + 1)
            ].T

    bgv = b_ih + b_hh
    bgT = np.stack(
        [
            gscale.get(g, 1.0) * bgv[g * H + HS * r : g * H + HS * (r + 1)]
            for g in GATE_ORDER
        ],
        axis=1,
    )  # [128, 4]

    WhT = np.zeros((128, 4096), f32)
    for x in range(8):
        srcc = src_row[x]
        for m, g in enumerate(GATE_ORDER):
            blk = W_hh[
                g * H + HS * r : g * H + HS * (r + 1), HS * srcc : HS * (srcc + 1)
            ]
            WhT[:, (4 * x + m) * 128 : (4 * x + m + 1) * 128] = gscale.get(
                g, 1.0
            ) * blk.T

    f32blob = np.concatenate(
        [
            np.ascontiguousarray(latent.T),
            np.ascontiguousarray(W_lin.T),
            np.ascontiguousarray(b_lin.reshape(8, 128).T),
            WihT,
            np.ascontiguousarray(bgT),
        ],
        axis=1,
    )
    b16blob = np.concatenate(
        [
            WhT.astype(bf16),
            np.ascontiguousarray(W_out[0, sl].reshape(128, 1)).astype(bf16),
            np.eye(128, dtype=f32).astype(bf16),
        ],
        axis=1,
    )
    return {"f32blob": f32blob, "b16blob": b16blob}


def _run(inputs: dict, trace: bool = False):
    from concourse.bass_utils import run_bass_kernel_spmd

    if "nc" not in _cache:
        _cache["nc"] = _build_lstm_nc(T)
    nc = _cache["nc"]
    in_maps = [_prep_core_inputs(inputs, r) for r in range(NC)]
    res = run_bass_kernel_spmd(
        nc, in_maps, core_ids=list(range(NC)), trace=trace
    )
    outs = [np.asarray(res.results[r]["outp"], np.float64) for r in range(NC)]
    b_out = np.asarray(inputs["b_out"], np.float64)
    total = outs[0]
    for o in outs[1:]:
        total = total + o
    total = total + b_out[0]
    out = total[:, :, None].astype(np.float32)
    return out, res


def kernel(**inputs) -> np.ndarray:
    seq_len = int(inputs.get("seq_len", T))
    assert seq_len == T, f"kernel hardcoded for seq_len={T}, got {seq_len}"
    out, _ = _run(inputs, trace=False)
    return out


# revision 17
# speedup vs baseline: 1.1350x; 1.0112x over previous
"""nn_Decoder (LSTM decoder) Trainium2 Bass kernel, 8-core tensor-parallel,
two phase-shifted batch streams.

Strategy (hardcoded for B=64, L=128, H=1024, O=1, T=256, 8 cores):
  The 4H=4096 gate rows are sharded 8 ways: each core owns a 128-row H-slice
  of each gate (layout f|g|i|o), computes gates transposed on PE (W_hh^T
  blocks stationary in bf16, h^T streamed), does the cell elementwise on
  ACT/DVE, and broadcasts its h^T chunk to all peers each step via one
  8-destination remote_dma_broadcast.

  The batch is split into two independent 32-wide streams (A = batch 0:32,
  B = 32:64) running the same recurrence phase-shifted by ~half a step.
  While stream A's h-broadcast is in flight, the engines process stream B's
  matmuls / cell, and vice versa.  Both streams' broadcast frames share
  SWDGE queue 0 in strict A,B alternation (a second SWDGE queue silently
  corrupts transfers on this runtime).

  v2 changes vs the 1.60 ms baseline (trace-driven):
  - The baseline ran the PE at K=4/8 HAM throttle (1.2 GHz) for 97% of the
    kernel: one >3.4us PE-idle window during the round-0/1 pipeline fill
    re-throttled it, and the steady state never has a 3.4us contiguous
    busy stretch to re-warm.  Rounds 0-1 now run big filler sections
    (FILL_EARLY) so the PE stays continuously busy through the pipeline
    fill and keeps the 2.4 GHz clock it earned during phase-1 matmuls.
  - x_gates is computed TRANSPOSED in phase 1b ([128 gate-rows, 64 batch]
    per gate block) and re-injected each round with a single N=128 matmul
    per (stream, hi/lo) using a constant identity stationary: 4 LDW+MM
    pairs per round instead of 16.
  - One [128,128] sigmoid per stream per round instead of 2x [128,64]
    (ACT ops are ~250ns fixed overhead + ~1 col/cycle).

  Other tricks carried over from the baseline:
  - Per-SOURCE arrival semaphores: each receiver's PE consumes h chunks as
    they arrive instead of waiting for all 8.
  - g-gate tanh folded into the gate sigmoid (host scales g rows by 2,
    DVE fixes up with *2-1).
  - Cell state + temporaries in SBUF.
  - x_gates re-injected into PSUM via identity matmuls (bf16 hi+lo split).
"""

import numpy as np
import ml_dtypes

B, L, H, O, T = 64, 128, 1024, 1, 256
NC = 8
NPH = 4
# device gate-column order f|g|i|o (indices into pytorch's i,f,g,o row blocks)
GATE_ORDER = [1, 2, 0, 3]
# per-stream gate slices in the [128, 128] gates tile (4 gates x 32 batch)
SG_F = slice(0, 32)
SG_G = slice(32, 64)
SG_I = slice(64, 96)
SG_O = slice(96, 128)
# Filler matmuls per stream section: keep the PE the (slightly) binding
# resource so rounds run PE-paced in lockstep across cores, and never let a
# PE-idle window reach the ~3.4us HAM MID threshold.
FILLER_N = 256
FILL_A = 1
FILL_B = 1
# Rounds 0-1 (pipeline fill) get large filler sections: the first h
# broadcasts take a full serial chain, and a single >3.4us PE-idle window
# there re-throttles the PE to 1.2 GHz for the whole kernel.
FILL_EARLY = 6

_cache = {}


def _build_lstm_nc(T_steps=T, solo=False, detect_races=True):
    import concourse.bacc as bacc
    import concourse.bass as bass
    import concourse.mybir as mybir

    dt = mybir.dt
    AF = mybir.ActivationFunctionType
    ALU = mybir.AluOpType
    Tn = T_steps

    nc = bacc.Bacc(
        None,
        target_bir_lowering=False,
        debug=False,
        num_devices=NC,
        detect_race_conditions=detect_races,
    )

    # all fp32 inputs packed as one [128, 5196] blob: latT|WlinT|blinT|WihT|bgT
    d_f32 = nc.dram_tensor("f32blob", [128, 5196], dt.float32, kind="ExternalInput")
    # all bf16 inputs packed as one [128, 4225] blob: WhT|wout|I128
    d_b16 = nc.dram_tensor("b16blob", [128, 4225], dt.bfloat16, kind="ExternalInput")
    d_out = nc.dram_tensor("outp", [64, Tn], dt.float32, kind="ExternalOutput")
    N_IN = 2

    s_f32 = nc.alloc_sbuf_tensor("s_f32", [128, 5196], dt.float32)
    s_latT = s_f32[:, 0:64]
    s_WlinT = s_f32[:, 64:1088]
    s_blinT = s_f32[:, 1088:1096]
    s_WihT = s_f32[:, 1096:5192]
    s_bgT = s_f32[:, 5192:5196]
    s_b16 = nc.alloc_sbuf_tensor("s_b16", [128, 4225], dt.bfloat16)
    s_WhT = s_b16[:, 0:4096]
    s_wout = s_b16[:, 4096:4097]
    s_I128 = s_b16[:, 4097:4225]

    s_hidT = nc.alloc_sbuf_tensor("s_hidT", [128, 512], dt.float32)
    # x_gates transposed: [128 gate-rows, gate-block m, batch] (hi/lo bf16)
    s_xT = nc.alloc_sbuf_tensor("s_xT", [128, 256], dt.float32)
    s_xres = nc.alloc_sbuf_tensor("s_xres", [128, 256], dt.float32)
    s_xThi = nc.alloc_sbuf_tensor("s_xThi", [128, 4, 64], dt.bfloat16)
    s_xTlo = nc.alloc_sbuf_tensor("s_xTlo", [128, 4, 64], dt.bfloat16)
    # per-stream recv buffers: 4-deep rotation, 8 slots x 32 batch cols
    recvA = [
        nc.alloc_sbuf_tensor(f"recvA{p}", [128, 256], dt.bfloat16) for p in range(NPH)
    ]
    recvB = [
        nc.alloc_sbuf_tensor(f"recvB{p}", [128, 256], dt.bfloat16) for p in range(NPH)
    ]
    gA = [nc.alloc_sbuf_tensor(f"gA{p}", [128, 128], dt.float32) for p in range(2)]
    gB = [nc.alloc_sbuf_tensor(f"gB{p}", [128, 128], dt.float32) for p in range(2)]
    thA = [nc.alloc_sbuf_tensor(f"thA{p}", [128, 32], dt.float32) for p in range(2)]
    thB = [nc.alloc_sbuf_tensor(f"thB{p}", [128, 32], dt.float32) for p in range(2)]
    # each send buffer padded to its own 512B-aligned footprint
    _hsA = [
        nc.alloc_sbuf_tensor(f"h_sendA{p}", [128, 256], dt.bfloat16) for p in range(2)
    ]
    _hsB = [
        nc.alloc_sbuf_tensor(f"h_sendB{p}", [128, 256], dt.bfloat16) for p in range(2)
    ]
    h_sendA = [t[:, 0:32] for t in _hsA]
    h_sendB = [t[:, 0:32] for t in _hsB]
    s_t1 = nc.alloc_sbuf_tensor("s_t1", [128, 32], dt.float32)
    s_t2 = nc.alloc_sbuf_tensor("s_t2", [128, 32], dt.float32)
    s_gt = nc.alloc_sbuf_tensor("s_gt", [128, 32], dt.float32)
    cA = [nc.alloc_sbuf_tensor(f"cA{p}", [128, 32], dt.float32) for p in range(2)]
    cB = [nc.alloc_sbuf_tensor(f"cB{p}", [128, 32], dt.float32) for p in range(2)]
    s_out = nc.alloc_sbuf_tensor("s_out", [64, Tn], dt.float32)

    p_hid = nc.alloc_psum_tensor("p_hid", [128, 512], dt.float32)
    p_x = nc.alloc_psum_tensor("p_x", [128, 512], dt.float32)
    p_gA = [
        nc.alloc_psum_tensor(f"p_gA{p}", [128, 512], dt.float32) for p in range(2)
    ]
    p_gB = [
        nc.alloc_psum_tensor(f"p_gB{p}", [128, 512], dt.float32) for p in range(2)
    ]
    p_out = nc.alloc_psum_tensor("p_out", [128, 512], dt.float32)
    p_fill = nc.alloc_psum_tensor("p_fill", [128, 512], dt.float32)

    s_srcA = [nc.alloc_semaphore(f"s_srcA{j}") for j in range(NC)]
    s_srcB = [nc.alloc_semaphore(f"s_srcB{j}") for j in range(NC)]
    s_peA = nc.alloc_semaphore("s_peA")
    s_peB = nc.alloc_semaphore("s_peB")
    s_sigA = nc.alloc_semaphore("s_sigA")
    s_sigB = nc.alloc_semaphore("s_sigB")
    s_thsA = nc.alloc_semaphore("s_thsA")
    s_thsB = nc.alloc_semaphore("s_thsB")
    s_cA = nc.alloc_semaphore("s_cA")
    s_cB = nc.alloc_semaphore("s_cB")
    s_hA = nc.alloc_semaphore("s_hA")
    s_hB = nc.alloc_semaphore("s_hB")
    s_locA = nc.alloc_semaphore("s_locA")
    s_locB = nc.alloc_semaphore("s_locB")
    s_prepA = nc.alloc_semaphore("s_prepA")
    s_prepB = nc.alloc_semaphore("s_prepB")
    s_ph = nc.alloc_semaphore("s_ph")
    s_xa = nc.alloc_semaphore("s_xa")
    s_v = nc.alloc_semaphore("s_v")
    s_xrdy = nc.alloc_semaphore("s_xrdy")
    s_osem = nc.alloc_semaphore("s_osem")
    s_fin = nc.alloc_semaphore("s_fin")
    dma_sem = nc.alloc_semaphore("dma_sem")

    SRC_INC = 16 if solo else 2

    # X-inject: pg[:, 0:128] = xT (hi then lo), via constant identity
    # stationary; the moving AP walks [4 gate blocks, 32 batch cols].
    # bslice selects the stream's batch half inside each 64-wide block.
    def x_inject(tensor, pg, bstart, final_stop=False):
        tensor.matmul(
            pg[:, 0:128],
            s_b16[:, 4097:4225],
            s_xThi[:, :, bstart : bstart + 32],
            start=True,
            stop=False,
        )
        mm = tensor.matmul(
            pg[:, 0:128],
            s_b16[:, 4097:4225],
            s_xTlo[:, :, bstart : bstart + 32],
            start=False,
            stop=final_stop,
        )
        return mm

    def fillers(tensor, n):
        for fi in range(n):
            tensor.matmul(
                p_fill[:, 0:FILLER_N],
                s_b16[:, 0:128],
                s_b16[:, 128 : 128 + FILLER_N],
                start=(fi == 0),
                stop=(fi == n - 1),
            )

    with nc.Block() as block:

        @block.sync
        def _(sync: bass.BassEngine):
            sync.dma_start(s_f32[:, :], d_f32[:, :]).then_inc(dma_sem, 16)
            sync.dma_start(s_b16[:, :], d_b16[:, :]).then_inc(dma_sem, 16)
            sync.wait_ge(s_fin, 1)
            sync.dma_start(d_out[:, :], s_out[:, :]).then_inc(dma_sem, 16)
            sync.wait_ge(dma_sem, 16 * (N_IN + 1))

        @block.tensor
        def _(tensor: bass.BassTensorEngine):
            tensor.wait_ge(dma_sem, 16 * N_IN)
            # phase 1a: hidden^T chunks = W_lin row-chunks @ latent^T
            for m in range(8):
                mm = tensor.matmul(
                    p_hid[:, 64 * m : 64 * m + 64],
                    s_f32[:, 64 + 128 * m : 64 + 128 * m + 128],
                    s_latT,
                    start=True,
                    stop=True,
                )
            mm.then_inc(s_ph, 1)  # s_ph = 1
            # HAM warmup BEFORE phase 1b so the fp32 1b matmuls run at
            # 2.4 GHz (also overlaps the ACT hidden-bias stage)
            for fi in range(12):
                tensor.matmul(
                    p_fill[:, 0:512],
                    s_b16[:, 0:128],
                    s_b16[:, 128:640],
                    start=(fi == 0),
                    stop=(fi == 11),
                )
            # phase 1b: x_gates TRANSPOSED: for each gate block m (f|g|i|o),
            # xT[:, 64m:64m+64] = Wih_m @ hidden^T, accumulated over 8
            # h-chunks k.  Stationary block (m,k) lives at col (m*8+k)*128.
            tensor.wait_ge(s_ph, 2)
            for m in range(4):
                for k in range(8):
                    mm = tensor.matmul(
                        p_x[:, 64 * m : 64 * m + 64],
                        s_f32[:, 1096 + (m * 8 + k) * 128 : 1096 + (m * 8 + k + 1) * 128],
                        s_hidT[:, 64 * k : 64 * k + 64],
                        start=(k == 0),
                        stop=(k == 7),
                    )
                mm.then_inc(s_ph, 1)  # s_ph = 3 + m
            # prologue: round-0 gates = X only
            tensor.wait_ge(s_xrdy, 1)
            x_inject(tensor, p_gA[0], 0, final_stop=True).then_inc(s_peA, 1)
            x_inject(tensor, p_gB[0], 32, final_stop=True).then_inc(s_peB, 1)

            for r in range(Tn):
                nfill_a = FILL_EARLY if r < 2 else FILL_A
                nfill_b = FILL_EARLY if r < 2 else FILL_B
                # ---- stream A ----
                if r >= 1:
                    par = r % NPH
                    pg = p_gA[r % 2]
                    for x in range(8):
                        tensor.wait_ge(s_srcA[x], SRC_INC * r)
                        for m in range(4):
                            mm = tensor.matmul(
                                pg[:, 32 * m : 32 * m + 32],
                                s_b16[:, (4 * x + m) * 128 : (4 * x + m + 1) * 128],
                                recvA[par][:, 32 * x : 32 * x + 32],
                                start=False,
                                stop=(x == 7 and m == 3),
                            )
                    mm.then_inc(s_peA, 1)  # r+1
                if r + 1 < Tn:
                    # X for round r+1 opens the pg[(r+1)%2] accumulation group
                    x_inject(tensor, p_gA[(r + 1) % 2], 0)
                if r >= 1:
                    tensor.wait_ge(s_hA, r)
                    tensor.matmul(
                        p_out[0:32, r - 1 : r],
                        h_sendA[r % 2],
                        s_b16[:, 4096:4097],
                        start=True,
                        stop=True,
                    )
                fillers(tensor, nfill_a)
                # ---- stream B ----
                if r >= 1:
                    par = r % NPH
                    pg = p_gB[r % 2]
                    for x in range(8):
                        tensor.wait_ge(s_srcB[x], SRC_INC * r)
                        for m in range(4):
                            mm = tensor.matmul(
                                pg[:, 32 * m : 32 * m + 32],
                                s_b16[:, (4 * x + m) * 128 : (4 * x + m + 1) * 128],
                                recvB[par][:, 32 * x : 32 * x + 32],
                                start=False,
                                stop=(x == 7 and m == 3),
                            )
                    mm.then_inc(s_peB, 1)  # r+1
                if r + 1 < Tn:
                    x_inject(tensor, p_gB[(r + 1) % 2], 32)
                if r >= 1:
                    tensor.wait_ge(s_hB, r)
                    tensor.matmul(
                        p_out[32:64, r - 1 : r],
                        h_sendB[r % 2],
                        s_b16[:, 4096:4097],
                        start=True,
                        stop=True,
                    )
                fillers(tensor, nfill_b)

            tensor.wait_ge(s_hA, Tn)
            tensor.matmul(
                p_out[0:32, Tn - 1 : Tn],
                h_sendA[Tn % 2],
                s_b16[:, 4096:4097],
                start=True,
                stop=True,
            ).then_inc(s_osem, 1)
            tensor.wait_ge(s_hB, Tn)
            tensor.matmul(
                p_out[32:64, Tn - 1 : Tn],
                h_sendB[Tn % 2],
                s_b16[:, 4096:4097],
                start=True,
                stop=True,
            ).then_inc(s_osem, 1)

        @block.scalar
        def _(scalar: bass.BassScalarEngine):
            scalar.wait_ge(s_ph, 1)
            for m in range(8):
                a = scalar.activation(
                    s_hidT[:, 64 * m : 64 * m + 64],
                    p_hid[:, 64 * m : 64 * m + 64],
                    AF.Identity,
                    bias=s_f32[:, 1088 + m : 1088 + m + 1],
                    scale=1.0,
                )
            a.then_inc(s_ph, 1)  # s_ph = 2
            # xT bias add per gate block (bias is per-partition here)
            # (wait for ALL four groups: reading a PSUM bank while the PE
            # still accumulates other columns of the same bank is unsafe)
            scalar.wait_ge(s_ph, 6)
            for m in range(4):
                scalar.activation(
                    s_xT[:, 64 * m : 64 * m + 64],
                    p_x[:, 64 * m : 64 * m + 64],
                    AF.Identity,
                    bias=s_f32[:, 5192 + m : 5192 + m + 1],
                    scale=1.0,
                ).then_inc(s_xa, 1)
            for r in range(Tn):
                scalar.wait_ge(s_peA, r + 1)
                scalar.activation(
                    gA[r % 2][:, 0:128], p_gA[r % 2][:, 0:128], AF.Sigmoid
                ).then_inc(s_sigA, 1)  # r+1
                scalar.wait_ge(s_cA, r + 1)
                scalar.activation(
                    thA[r % 2][:, :], cA[r % 2][:, :], AF.Tanh
                ).then_inc(s_thsA, 1)  # r+1
                scalar.wait_ge(s_peB, r + 1)
                scalar.activation(
                    gB[r % 2][:, 0:128], p_gB[r % 2][:, 0:128], AF.Sigmoid
                ).then_inc(s_sigB, 1)  # r+1
                scalar.wait_ge(s_cB, r + 1)
                scalar.activation(
                    thB[r % 2][:, :], cB[r % 2][:, :], AF.Tanh
                ).then_inc(s_thsB, 1)  # r+1
            scalar.wait_ge(s_osem, 2)
            scalar.activation(s_out[:, :], p_out[0:64, 0:Tn], AF.Copy).then_inc(
                s_fin, 1
            )

        @block.vector
        def _(vector: bass.BassVectorEngine):
            vector.wait_ge(s_xa, 4)
            vector.tensor_copy(s_xThi[:, :, :], s_xT[:, :]).then_inc(s_v, 1)
            vector.wait_ge(s_v, 1)
            vector.tensor_tensor(
                s_xres[:, :], s_xT[:, :], s_xThi[:, :, :], ALU.subtract
            ).then_inc(s_v, 1)
            vector.wait_ge(s_v, 2)
            vector.tensor_copy(s_xTlo[:, :, :], s_xres[:, :])
            vector.memset(cA[1][:, :], 0.0)
            vector.memset(cB[1][:, :], 0.0).then_inc(s_xrdy, 1)
            # intra-DVE RAW edges (gt->t2, t2->c) carry explicit self-sems:
            # back-to-back DVE ops can read an operand before the prior op's
            # write fully lands.
            for r in range(Tn):
                # ---- stream A cell ----
                g = gA[r % 2]
                vector.wait_ge(s_sigA, r + 1)
                vector.tensor_tensor(
                    s_t1[:, :], g[:, SG_F], cA[(r + 1) % 2][:, :], ALU.mult
                ).then_inc(s_v, 1)  # 6r+3
                vector.scalar_tensor_tensor(
                    s_t2[:, :], g[:, SG_G], -0.5, g[:, SG_I], ALU.add, ALU.mult
                ).then_inc(s_v, 2)  # 6r+5  (= t2/2)
                vector.wait_ge(s_v, 6 * r + 5)
                vector.scalar_tensor_tensor(
                    cA[r % 2][:, :], s_t2[:, :], 2.0, s_t1[:, :], ALU.mult, ALU.add
                ).then_inc(s_cA, 1)  # r+1
                vector.wait_ge(s_thsA, r + 1)
                if r >= 2 and not solo:
                    vector.wait_ge(s_locA, 16 * (r - 1))
                vector.tensor_tensor(
                    h_sendA[(r + 1) % 2], g[:, SG_O], thA[r % 2][:, :], ALU.mult
                ).then_inc(s_hA, 1)  # r+1
                # ---- stream B cell ----
                g = gB[r % 2]
                vector.wait_ge(s_sigB, r + 1)
                vector.tensor_tensor(
                    s_t1[:, :], g[:, SG_F], cB[(r + 1) % 2][:, :], ALU.mult
                ).then_inc(s_v, 1)  # 6r+6
                vector.scalar_tensor_tensor(
                    s_t2[:, :], g[:, SG_G], -0.5, g[:, SG_I], ALU.add, ALU.mult
                ).then_inc(s_v, 2)  # 6r+8  (= t2/2)
                vector.wait_ge(s_v, 6 * r + 8)
                vector.scalar_tensor_tensor(
                    cB[r % 2][:, :], s_t2[:, :], 2.0, s_t1[:, :], ALU.mult, ALU.add
                ).then_inc(s_cB, 1)  # r+1
                vector.wait_ge(s_thsB, r + 1)
                if r >= 2 and not solo:
                    vector.wait_ge(s_locB, 16 * (r - 1))
                vector.tensor_tensor(
                    h_sendB[(r + 1) % 2], g[:, SG_O], thB[r % 2][:, :], ALU.mult
                ).then_inc(s_hB, 1)  # r+1

        @block.gpsimd
        def _(gpsimd: bass.BassGpSimd):
            if solo:
                for r in range(Tn):
                    gpsimd.wait_ge(s_hA, r + 1)
                    for j in range(8):
                        gpsimd.dma_start(
                            recvA[(r + 1) % NPH][:, 32 * j : 32 * j + 32],
                            h_sendA[(r + 1) % 2],
                        ).then_inc(s_srcA[j], 16)
                    gpsimd.wait_ge(s_hB, r + 1)
                    for j in range(8):
                        gpsimd.dma_start(
                            recvB[(r + 1) % NPH][:, 32 * j : 32 * j + 32],
                            h_sendB[(r + 1) % 2],
                        ).then_inc(s_srcB[j], 16)
                return
            gpsimd.bir_kernel_barrier_wait([list(range(NC))])
            pid = gpsimd.partition_id()
            for case in gpsimd.Switch(pid, NC):
                rdests = [(0, j) for j in range(NC)]
                # prologue: prep round-0 frames (A then B, strict FIFO order)
                gpsimd.remote_dma_broadcast(
                    out_ap=recvA[1][:, 32 * case : 32 * case + 32],
                    in_ap=h_sendA[1],
                    remote_sem=s_srcA[case],
                    local_sem=s_locA,
                    rdests=rdests,
                ).then_inc(s_prepA, 1)
                gpsimd.remote_dma_broadcast(
                    out_ap=recvB[1][:, 32 * case : 32 * case + 32],
                    in_ap=h_sendB[1],
                    remote_sem=s_srcB[case],
                    local_sem=s_locB,
                    rdests=rdests,
                ).then_inc(s_prepB, 1)
                for r in range(Tn):
                    # prep(r+2) frames are emitted in the A,B ring order but
                    # BETWEEN the two triggers, so descriptor generation never
                    # delays a trigger whose h just became ready.
                    gpsimd.wait_ge(s_prepA, r + 1)
                    gpsimd.wait_ge(s_hA, r + 1)
                    gpsimd.trigger_dma(count=1)  # fires frame A(r)
                    if r + 1 < Tn:
                        gpsimd.remote_dma_broadcast(
                            out_ap=recvA[(r + 2) % NPH][
                                :, 32 * case : 32 * case + 32
                            ],
                            in_ap=h_sendA[(r + 2) % 2],
                            remote_sem=s_srcA[case],
                            local_sem=s_locA,
                            rdests=rdests,
                        ).then_inc(s_prepA, 1)
                    gpsimd.wait_ge(s_prepB, r + 1)
                    gpsimd.wait_ge(s_hB, r + 1)
                    gpsimd.trigger_dma(count=1)  # fires frame B(r)
                    if r + 1 < Tn:
                        gpsimd.remote_dma_broadcast(
                            out_ap=recvB[(r + 2) % NPH][
                                :, 32 * case : 32 * case + 32
                            ],
                            in_ap=h_sendB[(r + 2) % 2],
                            remote_sem=s_srcB[case],
                            local_sem=s_locB,
                            rdests=rdests,
                        ).then_inc(s_prepB, 1)
                    gpsimd.wait_ge(s_locA, 16 * r)
                    gpsimd.wait_ge(s_locB, 16 * r)

    nc.has_collectives = not solo
    nc.finalize()
    return nc


def _prep_core_inputs(inputs: dict, r: int, src_row=None) -> dict:
    if src_row is None:
        src_row = list(range(8))  # slot j holds logical core j's H-chunk
    f32 = np.float32
    bf16 = ml_dtypes.bfloat16
    latent = np.asarray(inputs["latent"], f32)
    W_lin = np.asarray(inputs["W_lin"], f32)
    b_lin = np.asarray(inputs["b_lin"], f32)
    W_ih = np.asarray(inputs["W_ih"], f32)
    W_hh = np.asarray(inputs["W_hh"], f32)
    b_ih = np.asarray(inputs["b_ih"], f32)
    b_hh = np.asarray(inputs["b_hh"], f32)
    W_out = np.asarray(inputs["W_out"], f32)

    HS = 128
    sl = slice(HS * r, HS * (r + 1))

    # g-gate (pytorch index 2) rows scaled by 2: tanh(x) = 2*sigmoid(2x)-1,
    # so the device applies one sigmoid to all four gates and DVE fixes g up.
    gscale = {2: 2.0}

    # WihT: stationary block (m, k) at cols (m*8+k)*128: Wih[gate block m,
    # h-chunk k]^T so the device can compute x_gates transposed.
    WihT = np.zeros((128, 4096), f32)
    for m, g in enumerate(GATE_ORDER):
        blk_rows = gscale.get(g, 1.0) * W_ih[g * H + HS * r : g * H + HS * (r + 1), :]
        for k in range(8):
            WihT[:, (m * 8 + k) * 128 : (m * 8 + k + 1) * 128] = blk_rows[
                :, HS * k : HS * (k + 1)
            ].T

    bgv = b_ih + b_hh
    bgT = np.stack(
        [
            gscale.get(g, 1.0) * bgv[g * H + HS * r : g * H + HS * (r + 1)]
            for g in GATE_ORDER
        ],
        axis=1,
    )  # [128, 4]

    WhT = np.zeros((128, 4096), f32)
    for x in range(8):
        srcc = src_row[x]
        for m, g in enumerate(GATE_ORDER):
            blk = W_hh[
                g * H + HS * r : g * H + HS * (r + 1), HS * srcc : HS * (srcc + 1)
            ]
            WhT[:, (4 * x + m) * 128 : (4 * x + m + 1) * 128] = gscale.get(
                g, 1.0
            ) * blk.T

    f32blob = np.concatenate(
        [
            np.ascontiguousarray(latent.T),
            np.ascontiguousarray(W_lin.T),
            np.ascontiguousarray(b_lin.reshape(8, 128).T),
            WihT,
            np.ascontiguousarray(bgT),
        ],
        axis=1,
    )
    b16blob = np.concatenate(
        [
            WhT.astype(bf16),
            np.ascontiguousarray(W_out[0, sl].reshape(128, 1)).astype(bf16),
            np.eye(128, dtype=f32).astype(bf16),
        ],
        axis=1,
    )
    return {"f32blob": f32blob, "b16blob": b16blob}


def _run(inputs: dict, trace: bool = False):
    from concourse.bass_utils import run_bass_kernel_spmd

    if "nc" not in _cache:
        _cache["nc"] = _build_lstm_nc(T)
    nc = _cache["nc"]
    in_maps = [_prep_core_inputs(inputs, r) for r in range(NC)]
    res = run_bass_kernel_spmd(
        nc, in_maps, core_ids=list(range(NC)), trace=trace
    )
    outs = [np.asarray(res.results[r]["outp"], np.float64) for r in range(NC)]
    b_out = np.asarray(inputs["b_out"], np.float64)
    total = outs[0]
    for o in outs[1:]:
        total = total + o
    total = total + b_out[0]
    out = total[:, :, None].astype(np.float32)
    return out, res


def kernel(**inputs) -> np.ndarray:
    seq_len = int(inputs.get("seq_len", T))
    assert seq_len == T, f"kernel hardcoded for seq_len={T}, got {seq_len}"
    out, _ = _run(inputs, trace=False)
    return out


# revision 18
# speedup vs baseline: 1.1382x; 1.0028x over previous
"""nn_Decoder (LSTM decoder) Trainium2 Bass kernel, 8-core tensor-parallel,
two phase-shifted batch streams.

Strategy (hardcoded for B=64, L=128, H=1024, O=1, T=256, 8 cores):
  The 4H=4096 gate rows are sharded 8 ways: each core owns a 128-row H-slice
  of each gate (layout f|g|i|o), computes gates transposed on PE (W_hh^T
  blocks stationary in bf16, h^T streamed), does the cell elementwise on
  ACT/DVE, and broadcasts its h^T chunk to all peers each step via one
  8-destination remote_dma_broadcast.

  The batch is split into two independent 32-wide streams (A = batch 0:32,
  B = 32:64) running the same recurrence phase-shifted by ~half a step.
  While stream A's h-broadcast is in flight, the engines process stream B's
  matmuls / cell, and vice versa.  Both streams' broadcast frames share
  SWDGE queue 0 in strict A,B alternation (a second SWDGE queue silently
  corrupts transfers on this runtime).

  v2 changes vs the 1.60 ms baseline (trace-driven):
  - The baseline ran the PE at K=4/8 HAM throttle (1.2 GHz) for 97% of the
    kernel: one >3.4us PE-idle window during the round-0/1 pipeline fill
    re-throttled it, and the steady state never has a 3.4us contiguous
    busy stretch to re-warm.  Rounds 0-1 now run big filler sections
    (FILL_EARLY) so the PE stays continuously busy through the pipeline
    fill and keeps the 2.4 GHz clock it earned during phase-1 matmuls.
  - x_gates is computed TRANSPOSED in phase 1b ([128 gate-rows, 64 batch]
    per gate block) and re-injected each round with a single N=128 matmul
    per (stream, hi/lo) using a constant identity stationary: 4 LDW+MM
    pairs per round instead of 16.
  - One [128,128] sigmoid per stream per round instead of 2x [128,64]
    (ACT ops are ~250ns fixed overhead + ~1 col/cycle).

  Other tricks carried over from the baseline:
  - Per-SOURCE arrival semaphores: each receiver's PE consumes h chunks as
    they arrive instead of waiting for all 8.
  - g-gate tanh folded into the gate sigmoid (host scales g rows by 2,
    DVE fixes up with *2-1).
  - Cell state + temporaries in SBUF.
  - x_gates re-injected into PSUM via identity matmuls (bf16 hi+lo split).
"""

import numpy as np
import ml_dtypes

B, L, H, O, T = 64, 128, 1024, 1, 256
NC = 8
NPH = 4
# device gate-column order f|g|i|o (indices into pytorch's i,f,g,o row blocks)
GATE_ORDER = [1, 2, 0, 3]
# per-stream gate slices in the [128, 128] gates tile (4 gates x 32 batch)
SG_F = slice(0, 32)
SG_G = slice(32, 64)
SG_I = slice(64, 96)
SG_O = slice(96, 128)
# Filler matmuls per stream section: keep the PE the (slightly) binding
# resource so rounds run PE-paced in lockstep across cores, and never let a
# PE-idle window reach the ~3.4us HAM MID threshold.
FILLER_N = 256
FILL_A = 0
FILL_B = 0
# Rounds 0-1 (pipeline fill) get large filler sections: the first h
# broadcasts take a full serial chain, and a single >3.4us PE-idle window
# there re-throttles the PE to 1.2 GHz for the whole kernel.
FILL_EARLY = 6

_cache = {}


def _build_lstm_nc(T_steps=T, solo=False, detect_races=True):
    import concourse.bacc as bacc
    import concourse.bass as bass
    import concourse.mybir as mybir

    dt = mybir.dt
    AF = mybir.ActivationFunctionType
    ALU = mybir.AluOpType
    Tn = T_steps

    nc = bacc.Bacc(
        None,
        target_bir_lowering=False,
        debug=False,
        num_devices=NC,
        detect_race_conditions=detect_races,
    )

    # all fp32 inputs packed as one [128, 5196] blob: latT|WlinT|blinT|WihT|bgT
    d_f32 = nc.dram_tensor("f32blob", [128, 5196], dt.float32, kind="ExternalInput")
    # all bf16 inputs packed as one [128, 4225] blob: WhT|wout|I128
    d_b16 = nc.dram_tensor("b16blob", [128, 4225], dt.bfloat16, kind="ExternalInput")
    d_out = nc.dram_tensor("outp", [64, Tn], dt.float32, kind="ExternalOutput")
    N_IN = 2

    s_f32 = nc.alloc_sbuf_tensor("s_f32", [128, 5196], dt.float32)
    s_latT = s_f32[:, 0:64]
    s_WlinT = s_f32[:, 64:1088]
    s_blinT = s_f32[:, 1088:1096]
    s_WihT = s_f32[:, 1096:5192]
    s_bgT = s_f32[:, 5192:5196]
    s_b16 = nc.alloc_sbuf_tensor("s_b16", [128, 4225], dt.bfloat16)
    s_WhT = s_b16[:, 0:4096]
    s_wout = s_b16[:, 4096:4097]
    s_I128 = s_b16[:, 4097:4225]

    s_hidT = nc.alloc_sbuf_tensor("s_hidT", [128, 512], dt.float32)
    # x_gates transposed: [128 gate-rows, gate-block m, batch] (hi/lo bf16)
    s_xT = nc.alloc_sbuf_tensor("s_xT", [128, 256], dt.float32)
    s_xres = nc.alloc_sbuf_tensor("s_xres", [128, 256], dt.float32)
    s_xThi = nc.alloc_sbuf_tensor("s_xThi", [128, 4, 64], dt.bfloat16)
    s_xTlo = nc.alloc_sbuf_tensor("s_xTlo", [128, 4, 64], dt.bfloat16)
    # per-stream recv buffers: 4-deep rotation, 8 slots x 32 batch cols
    recvA = [
        nc.alloc_sbuf_tensor(f"recvA{p}", [128, 256], dt.bfloat16) for p in range(NPH)
    ]
    recvB = [
        nc.alloc_sbuf_tensor(f"recvB{p}", [128, 256], dt.bfloat16) for p in range(NPH)
    ]
    gA = [nc.alloc_sbuf_tensor(f"gA{p}", [128, 128], dt.float32) for p in range(2)]
    gB = [nc.alloc_sbuf_tensor(f"gB{p}", [128, 128], dt.float32) for p in range(2)]
    thA = [nc.alloc_sbuf_tensor(f"thA{p}", [128, 32], dt.float32) for p in range(2)]
    thB = [nc.alloc_sbuf_tensor(f"thB{p}", [128, 32], dt.float32) for p in range(2)]
    # each send buffer padded to its own 512B-aligned footprint
    _hsA = [
        nc.alloc_sbuf_tensor(f"h_sendA{p}", [128, 256], dt.bfloat16) for p in range(2)
    ]
    _hsB = [
        nc.alloc_sbuf_tensor(f"h_sendB{p}", [128, 256], dt.bfloat16) for p in range(2)
    ]
    h_sendA = [t[:, 0:32] for t in _hsA]
    h_sendB = [t[:, 0:32] for t in _hsB]
    s_t1 = nc.alloc_sbuf_tensor("s_t1", [128, 32], dt.float32)
    s_t2 = nc.alloc_sbuf_tensor("s_t2", [128, 32], dt.float32)
    s_gt = nc.alloc_sbuf_tensor("s_gt", [128, 32], dt.float32)
    cA = [nc.alloc_sbuf_tensor(f"cA{p}", [128, 32], dt.float32) for p in range(2)]
    cB = [nc.alloc_sbuf_tensor(f"cB{p}", [128, 32], dt.float32) for p in range(2)]
    s_out = nc.alloc_sbuf_tensor("s_out", [64, Tn], dt.float32)

    p_hid = nc.alloc_psum_tensor("p_hid", [128, 512], dt.float32)
    p_x = nc.alloc_psum_tensor("p_x", [128, 512], dt.float32)
    p_gA = [
        nc.alloc_psum_tensor(f"p_gA{p}", [128, 512], dt.float32) for p in range(2)
    ]
    p_gB = [
        nc.alloc_psum_tensor(f"p_gB{p}", [128, 512], dt.float32) for p in range(2)
    ]
    p_out = nc.alloc_psum_tensor("p_out", [128, 512], dt.float32)
    p_fill = nc.alloc_psum_tensor("p_fill", [128, 512], dt.float32)

    s_srcA = [nc.alloc_semaphore(f"s_srcA{j}") for j in range(NC)]
    s_srcB = [nc.alloc_semaphore(f"s_srcB{j}") for j in range(NC)]
    s_peA = nc.alloc_semaphore("s_peA")
    s_peB = nc.alloc_semaphore("s_peB")
    s_sigA = nc.alloc_semaphore("s_sigA")
    s_sigB = nc.alloc_semaphore("s_sigB")
    s_thsA = nc.alloc_semaphore("s_thsA")
    s_thsB = nc.alloc_semaphore("s_thsB")
    s_cA = nc.alloc_semaphore("s_cA")
    s_cB = nc.alloc_semaphore("s_cB")
    s_hA = nc.alloc_semaphore("s_hA")
    s_hB = nc.alloc_semaphore("s_hB")
    s_locA = nc.alloc_semaphore("s_locA")
    s_locB = nc.alloc_semaphore("s_locB")
    s_prepA = nc.alloc_semaphore("s_prepA")
    s_prepB = nc.alloc_semaphore("s_prepB")
    s_ph = nc.alloc_semaphore("s_ph")
    s_xa = nc.alloc_semaphore("s_xa")
    s_v = nc.alloc_semaphore("s_v")
    s_xrdy = nc.alloc_semaphore("s_xrdy")
    s_osem = nc.alloc_semaphore("s_osem")
    s_fin = nc.alloc_semaphore("s_fin")
    dma_sem = nc.alloc_semaphore("dma_sem")

    SRC_INC = 16 if solo else 2

    # X-inject: pg[:, 0:128] = xT (hi then lo), via constant identity
    # stationary; the moving AP walks [4 gate blocks, 32 batch cols].
    # bslice selects the stream's batch half inside each 64-wide block.
    def x_inject(tensor, pg, bstart, final_stop=False):
        tensor.matmul(
            pg[:, 0:128],
            s_b16[:, 4097:4225],
            s_xThi[:, :, bstart : bstart + 32],
            start=True,
            stop=False,
        )
        mm = tensor.matmul(
            pg[:, 0:128],
            s_b16[:, 4097:4225],
            s_xTlo[:, :, bstart : bstart + 32],
            start=False,
            stop=final_stop,
        )
        return mm

    def fillers(tensor, n):
        for fi in range(n):
            tensor.matmul(
                p_fill[:, 0:FILLER_N],
                s_b16[:, 0:128],
                s_b16[:, 128 : 128 + FILLER_N],
                start=(fi == 0),
                stop=(fi == n - 1),
            )

    with nc.Block() as block:

        @block.sync
        def _(sync: bass.BassEngine):
            sync.dma_start(s_f32[:, :], d_f32[:, :]).then_inc(dma_sem, 16)
            sync.dma_start(s_b16[:, :], d_b16[:, :]).then_inc(dma_sem, 16)
            sync.wait_ge(s_fin, 1)
            sync.dma_start(d_out[:, :], s_out[:, :]).then_inc(dma_sem, 16)
            sync.wait_ge(dma_sem, 16 * (N_IN + 1))

        @block.tensor
        def _(tensor: bass.BassTensorEngine):
            tensor.wait_ge(dma_sem, 16 * N_IN)
            # phase 1a: hidden^T chunks = W_lin row-chunks @ latent^T
            for m in range(8):
                mm = tensor.matmul(
                    p_hid[:, 64 * m : 64 * m + 64],
                    s_f32[:, 64 + 128 * m : 64 + 128 * m + 128],
                    s_latT,
                    start=True,
                    stop=True,
                )
            mm.then_inc(s_ph, 1)  # s_ph = 1
            # HAM warmup BEFORE phase 1b so the fp32 1b matmuls run at
            # 2.4 GHz (also overlaps the ACT hidden-bias stage)
            for fi in range(12):
                tensor.matmul(
                    p_fill[:, 0:512],
                    s_b16[:, 0:128],
                    s_b16[:, 128:640],
                    start=(fi == 0),
                    stop=(fi == 11),
                )
            # phase 1b: x_gates TRANSPOSED: for each gate block m (f|g|i|o),
            # xT[:, 64m:64m+64] = Wih_m @ hidden^T, accumulated over 8
            # h-chunks k.  Stationary block (m,k) lives at col (m*8+k)*128.
            tensor.wait_ge(s_ph, 2)
            for m in range(4):
                for k in range(8):
                    mm = tensor.matmul(
                        p_x[:, 64 * m : 64 * m + 64],
                        s_f32[:, 1096 + (m * 8 + k) * 128 : 1096 + (m * 8 + k + 1) * 128],
                        s_hidT[:, 64 * k : 64 * k + 64],
                        start=(k == 0),
                        stop=(k == 7),
                    )
                mm.then_inc(s_ph, 1)  # s_ph = 3 + m
            # prologue: round-0 gates = X only
            tensor.wait_ge(s_xrdy, 1)
            x_inject(tensor, p_gA[0], 0, final_stop=True).then_inc(s_peA, 1)
            x_inject(tensor, p_gB[0], 32, final_stop=True).then_inc(s_peB, 1)

            for r in range(Tn):
                nfill_a = FILL_EARLY if r < 2 else FILL_A
                nfill_b = FILL_EARLY if r < 2 else FILL_B
                # ---- stream A ----
                if r >= 1:
                    par = r % NPH
                    pg = p_gA[r % 2]
                    for x in range(8):
                        tensor.wait_ge(s_srcA[x], SRC_INC * r)
                        for m in range(4):
                            mm = tensor.matmul(
                                pg[:, 32 * m : 32 * m + 32],
                                s_b16[:, (4 * x + m) * 128 : (4 * x + m + 1) * 128],
                                recvA[par][:, 32 * x : 32 * x + 32],
                                start=False,
                                stop=(x == 7 and m == 3),
                            )
                    mm.then_inc(s_peA, 1)  # r+1
                if r + 1 < Tn:
                    # X for round r+1 opens the pg[(r+1)%2] accumulation group
                    x_inject(tensor, p_gA[(r + 1) % 2], 0)
                if r >= 1:
                    tensor.wait_ge(s_hA, r)
                    tensor.matmul(
                        p_out[0:32, r - 1 : r],
                        h_sendA[r % 2],
                        s_b16[:, 4096:4097],
                        start=True,
                        stop=True,
                    )
                fillers(tensor, nfill_a)
                # ---- stream B ----
                if r >= 1:
                    par = r % NPH
                    pg = p_gB[r % 2]
                    for x in range(8):
                        tensor.wait_ge(s_srcB[x], SRC_INC * r)
                        for m in range(4):
                            mm = tensor.matmul(
                                pg[:, 32 * m : 32 * m + 32],
                                s_b16[:, (4 * x + m) * 128 : (4 * x + m + 1) * 128],
                                recvB[par][:, 32 * x : 32 * x + 32],
                                start=False,
                                stop=(x == 7 and m == 3),
                            )
                    mm.then_inc(s_peB, 1)  # r+1
                if r + 1 < Tn:
                    x_inject(tensor, p_gB[(r + 1) % 2], 32)
                if r >= 1:
                    tensor.wait_ge(s_hB, r)
                    tensor.matmul(
                        p_out[32:64, r - 1 : r],
                        h_sendB[r % 2],
                        s_b16[:, 4096:4097],
                        start=True,
                        stop=True,
                    )
                fillers(tensor, nfill_b)

            tensor.wait_ge(s_hA, Tn)
            tensor.matmul(
                p_out[0:32, Tn - 1 : Tn],
                h_sendA[Tn % 2],
                s_b16[:, 4096:4097],
                start=True,
                stop=True,
            ).then_inc(s_osem, 1)
            tensor.wait_ge(s_hB, Tn)
            tensor.matmul(
                p_out[32:64, Tn - 1 : Tn],
                h_sendB[Tn % 2],
                s_b16[:, 4096:4097],
                start=True,
                stop=True,
            ).then_inc(s_osem, 1)

        @block.scalar
        def _(scalar: bass.BassScalarEngine):
            scalar.wait_ge(s_ph, 1)
            for m in range(8):
                a = scalar.activation(
                    s_hidT[:, 64 * m : 64 * m + 64],
                    p_hid[:, 64 * m : 64 * m + 64],
                    AF.Identity,
                    bias=s_f32[:, 1088 + m : 1088 + m + 1],
                    scale=1.0,
                )
            a.then_inc(s_ph, 1)  # s_ph = 2
            # xT bias add per gate block (bias is per-partition here)
            # (wait for ALL four groups: reading a PSUM bank while the PE
            # still accumulates other columns of the same bank is unsafe)
            scalar.wait_ge(s_ph, 6)
            for m in range(4):
                scalar.activation(
                    s_xT[:, 64 * m : 64 * m + 64],
                    p_x[:, 64 * m : 64 * m + 64],
                    AF.Identity,
                    bias=s_f32[:, 5192 + m : 5192 + m + 1],
                    scale=1.0,
                ).then_inc(s_xa, 1)
            for r in range(Tn):
                scalar.wait_ge(s_peA, r + 1)
                scalar.activation(
                    gA[r % 2][:, 0:128], p_gA[r % 2][:, 0:128], AF.Sigmoid
                ).then_inc(s_sigA, 1)  # r+1
                scalar.wait_ge(s_cA, r + 1)
                scalar.activation(
                    thA[r % 2][:, :], cA[r % 2][:, :], AF.Tanh
                ).then_inc(s_thsA, 1)  # r+1
                scalar.wait_ge(s_peB, r + 1)
                scalar.activation(
                    gB[r % 2][:, 0:128], p_gB[r % 2][:, 0:128], AF.Sigmoid
                ).then_inc(s_sigB, 1)  # r+1
                scalar.wait_ge(s_cB, r + 1)
                scalar.activation(
                    thB[r % 2][:, :], cB[r % 2][:, :], AF.Tanh
                ).then_inc(s_thsB, 1)  # r+1
            scalar.wait_ge(s_osem, 2)
            scalar.activation(s_out[:, :], p_out[0:64, 0:Tn], AF.Copy).then_inc(
                s_fin, 1
            )

        @block.vector
        def _(vector: bass.BassVectorEngine):
            vector.wait_ge(s_xa, 4)
            vector.tensor_copy(s_xThi[:, :, :], s_xT[:, :]).then_inc(s_v, 1)
            vector.wait_ge(s_v, 1)
            vector.tensor_tensor(
                s_xres[:, :], s_xT[:, :], s_xThi[:, :, :], ALU.subtract
            ).then_inc(s_v, 1)
            vector.wait_ge(s_v, 2)
            vector.tensor_copy(s_xTlo[:, :, :], s_xres[:, :])
            vector.memset(cA[1][:, :], 0.0)
            vector.memset(cB[1][:, :], 0.0).then_inc(s_xrdy, 1)
            # intra-DVE RAW edges (gt->t2, t2->c) carry explicit self-sems:
            # back-to-back DVE ops can read an operand before the prior op's
            # write fully lands.
            for r in range(Tn):
                # ---- stream A cell ----
                g = gA[r % 2]
                vector.wait_ge(s_sigA, r + 1)
                vector.tensor_tensor(
                    s_t1[:, :], g[:, SG_F], cA[(r + 1) % 2][:, :], ALU.mult
                ).then_inc(s_v, 1)  # 6r+3
                vector.scalar_tensor_tensor(
                    s_t2[:, :], g[:, SG_G], -0.5, g[:, SG_I], ALU.add, ALU.mult
                ).then_inc(s_v, 2)  # 6r+5  (= t2/2)
                vector.wait_ge(s_v, 6 * r + 5)
                vector.scalar_tensor_tensor(
                    cA[r % 2][:, :], s_t2[:, :], 2.0, s_t1[:, :], ALU.mult, ALU.add
                ).then_inc(s_cA, 1)  # r+1
                vector.wait_ge(s_thsA, r + 1)
                if r >= 2 and not solo:
                    vector.wait_ge(s_locA, 16 * (r - 1))
                vector.tensor_tensor(
                    h_sendA[(r + 1) % 2], g[:, SG_O], thA[r % 2][:, :], ALU.mult
                ).then_inc(s_hA, 1)  # r+1
                # ---- stream B cell ----
                g = gB[r % 2]
                vector.wait_ge(s_sigB, r + 1)
                vector.tensor_tensor(
                    s_t1[:, :], g[:, SG_F], cB[(r + 1) % 2][:, :], ALU.mult
                ).then_inc(s_v, 1)  # 6r+6
                vector.scalar_tensor_tensor(
                    s_t2[:, :], g[:, SG_G], -0.5, g[:, SG_I], ALU.add, ALU.mult
                ).then_inc(s_v, 2)  # 6r+8  (= t2/2)
                vector.wait_ge(s_v, 6 * r + 8)
                vector.scalar_tensor_tensor(
                    cB[r % 2][:, :], s_t2[:, :], 2.0, s_t1[:, :], ALU.mult, ALU.add
                ).then_inc(s_cB, 1)  # r+1
                vector.wait_ge(s_thsB, r + 1)
                if r >= 2 and not solo:
                    vector.wait_ge(s_locB, 16 * (r - 1))
                vector.tensor_tensor(
                    h_sendB[(r + 1) % 2], g[:, SG_O], thB[r % 2][:, :], ALU.mult
                ).then_inc(s_hB, 1)  # r+1

        @block.gpsimd
        def _(gpsimd: bass.BassGpSimd):
            if solo:
                for r in range(Tn):
                    gpsimd.wait_ge(s_hA, r + 1)
                    for j in range(8):
                        gpsimd.dma_start(
                            recvA[(r + 1) % NPH][:, 32 * j : 32 * j + 32],
                            h_sendA[(r + 1) % 2],
                        ).then_inc(s_srcA[j], 16)
                    gpsimd.wait_ge(s_hB, r + 1)
                    for j in range(8):
                        gpsimd.dma_start(
                            recvB[(r + 1) % NPH][:, 32 * j : 32 * j + 32],
                            h_sendB[(r + 1) % 2],
                        ).then_inc(s_srcB[j], 16)
                return
            gpsimd.bir_kernel_barrier_wait([list(range(NC))])
            pid = gpsimd.partition_id()
            for case in gpsimd.Switch(pid, NC):
                rdests = [(0, j) for j in range(NC)]
                # prologue: prep round-0 frames (A then B, strict FIFO order)
                gpsimd.remote_dma_broadcast(
                    out_ap=recvA[1][:, 32 * case : 32 * case + 32],
                    in_ap=h_sendA[1],
                    remote_sem=s_srcA[case],
                    local_sem=s_locA,
                    rdests=rdests,
                ).then_inc(s_prepA, 1)
                gpsimd.remote_dma_broadcast(
                    out_ap=recvB[1][:, 32 * case : 32 * case + 32],
                    in_ap=h_sendB[1],
                    remote_sem=s_srcB[case],
                    local_sem=s_locB,
                    rdests=rdests,
                ).then_inc(s_prepB, 1)
                for r in range(Tn):
                    # prep(r+2) frames are emitted in the A,B ring order but
                    # BETWEEN the two triggers, so descriptor generation never
                    # delays a trigger whose h just became ready.
                    gpsimd.wait_ge(s_prepA, r + 1)
                    gpsimd.wait_ge(s_hA, r + 1)
                    gpsimd.trigger_dma(count=1)  # fires frame A(r)
                    if r + 1 < Tn:
                        gpsimd.remote_dma_broadcast(
                            out_ap=recvA[(r + 2) % NPH][
                                :, 32 * case : 32 * case + 32
                            ],
                            in_ap=h_sendA[(r + 2) % 2],
                            remote_sem=s_srcA[case],
                            local_sem=s_locA,
                            rdests=rdests,
                        ).then_inc(s_prepA, 1)
                    gpsimd.wait_ge(s_prepB, r + 1)
                    gpsimd.wait_ge(s_hB, r + 1)
                    gpsimd.trigger_dma(count=1)  # fires frame B(r)
                    if r + 1 < Tn:
                        gpsimd.remote_dma_broadcast(
                            out_ap=recvB[(r + 2) % NPH][
                                :, 32 * case : 32 * case + 32
                            ],
                            in_ap=h_sendB[(r + 2) % 2],
                            remote_sem=s_srcB[case],
                            local_sem=s_locB,
                            rdests=rdests,
                        ).then_inc(s_prepB, 1)
                    gpsimd.wait_ge(s_locA, 16 * r)
                    gpsimd.wait_ge(s_locB, 16 * r)

    nc.has_collectives = not solo
    nc.finalize()
    return nc


def _prep_core_inputs(inputs: dict, r: int, src_row=None) -> dict:
    if src_row is None:
        src_row = list(range(8))  # slot j holds logical core j's H-chunk
    f32 = np.float32
    bf16 = ml_dtypes.bfloat16
    latent = np.asarray(inputs["latent"], f32)
    W_lin = np.asarray(inputs["W_lin"], f32)
    b_lin = np.asarray(inputs["b_lin"], f32)
    W_ih = np.asarray(inputs["W_ih"], f32)
    W_hh = np.asarray(inputs["W_hh"], f32)
    b_ih = np.asarray(inputs["b_ih"], f32)
    b_hh = np.asarray(inputs["b_hh"], f32)
    W_out = np.asarray(inputs["W_out"], f32)

    HS = 128
    sl = slice(HS * r, HS * (r + 1))

    # g-gate (pytorch index 2) rows scaled by 2: tanh(x) = 2*sigmoid(2x)-1,
    # so the device applies one sigmoid to all four gates and DVE fixes g up.
    gscale = {2: 2.0}

    # WihT: stationary block (m, k) at cols (m*8+k)*128: Wih[gate block m,
    # h-chunk k]^T so the device can compute x_gates transposed.
    WihT = np.zeros((128, 4096), f32)
    for m, g in enumerate(GATE_ORDER):
        blk_rows = gscale.get(g, 1.0) * W_ih[g * H + HS * r : g * H + HS * (r + 1), :]
        for k in range(8):
            WihT[:, (m * 8 + k) * 128 : (m * 8 + k + 1) * 128] = blk_rows[
                :, HS * k : HS * (k + 1)
            ].T

    bgv = b_ih + b_hh
    bgT = np.stack(
        [
            gscale.get(g, 1.0) * bgv[g * H + HS * r : g * H + HS * (r + 1)]
            for g in GATE_ORDER
        ],
        axis=1,
    )  # [128, 4]

    WhT = np.zeros((128, 4096), f32)
    for x in range(8):
        srcc = src_row[x]
        for m, g in enumerate(GATE_ORDER):
            blk = W_hh[
                g * H + HS * r : g * H + HS * (r + 1), HS * srcc : HS * (srcc + 1)
            ]
            WhT[:, (4 * x + m) * 128 : (4 * x + m + 1) * 128] = gscale.get(
                g, 1.0
            ) * blk.T

    f32blob = np.concatenate(
        [
            np.ascontiguousarray(latent.T),
            np.ascontiguousarray(W_lin.T),
            np.ascontiguousarray(b_lin.reshape(8, 128).T),
            WihT,
            np.ascontiguousarray(bgT),
        ],
        axis=1,
    )
    b16blob = np.concatenate(
        [
            WhT.astype(bf16),
            np.ascontiguousarray(W_out[0, sl].reshape(128, 1)).astype(bf16),
            np.eye(128, dtype=f32).astype(bf16),
        ],
        axis=1,
    )
    return {"f32blob": f32blob, "b16blob": b16blob}


def _run(inputs: dict, trace: bool = False):
    from concourse.bass_utils import run_bass_kernel_spmd

    if "nc" not in _cache:
        _cache["nc"] = _build_lstm_nc(T)
    nc = _cache["nc"]
    in_maps = [_prep_core_inputs(inputs, r) for r in range(NC)]
    res = run_bass_kernel_spmd(
        nc, in_maps, core_ids=list(range(NC)), trace=trace
    )
    outs = [np.asarray(res.results[r]["outp"], np.float64) for r in range(NC)]
    b_out = np.asarray(inputs["b_out"], np.float64)
    total = outs[0]
    for o in outs[1:]:
        total = total + o
    total = total + b_out[0]
    out = total[:, :, None].astype(np.float32)
    return out, res


def kernel(**inputs) -> np.ndarray:
    seq_len = int(inputs.get("seq_len", T))
    assert seq_len == T, f"kernel hardcoded for seq_len={T}, got {seq_len}"
    out, _ = _run(inputs, trace=False)
    return out
